# revision 8
# baseline (speedup 1.0000x reference)
"""Trainium2 Bass kernel for a 2-layer GAT (nn_GAT_781684048444).

Strategy (8 NeuronCores, SPMD) — v2:
  - Nodes assigned to 80 windows (8 cores x 10 windows x 128 slots) by greedy
    in-degree balancing; edges grouped by dst window, padded to K 128-edge
    tiles; one static program serves all cores. Output inverse-permuted.
  - Stage 0 (replicated): ONE bf16 matmul per 128-node block computes the
    packed row [512 bf16 msg (head-minor) | 8 fp32 s_src | 8 fp32 s_dst]
    (attention dots folded into the weight matrix; the fp32 PSUM result is
    staged to SBUF once, msg cols reach DRAM through a gpsimd cast-DMA
    (fp32->bf16 in the DMA), scalar cols through a raw bit-copy DMA on SP).
    No separate fp32 x load, no second matmul, no big DVE copy traffic.
  - Layer-1 edge phase: gather 1280B rows by src + 256B scalar blocks by dst;
    alpha adds run on gpsimd, leaky-relu on DVE, exp on ACT; the bf16
    msg *= alpha broadcast runs in DVE 2x mode; scatter-add via resident
    one-hot selT bf16 matmuls into PSUM. h1 = elu(U/D) with the elu expressed
    as relu/exp on ACT (elu(x) = relu(x) + exp(-relu(-x)) - 1), only the
    final combine on DVE. h1 kept bf16.
  - Layer-2 projection per window (bf16 transpose + matmul) feeds a compact
    [NPAD, 8] fp32 table; the AllGather is SPLIT in two (windows 0-4 issued
    mid-layer-1, 5-9 at the end) so most of its fixed cost overlaps layer 1.
    dst-side gathers + alpha partials are computed in the shadow of the
    second collective; src gathers + the final gather-attend-scatter follow.
"""

import os
import sys

import ml_dtypes
import numpy as np

sys.path.insert(0, "/opt/trn_rl_repo")

from concourse import bacc, bass, mybir, tile  # noqa: E402
from concourse.bass import AP  # noqa: E402
from concourse.bass_utils import run_bass_kernel_spmd  # noqa: E402

N, E = 10000, 160000
IN, HID, OUT, H = 128, 64, 4, 8
C1 = H * HID               # 512 layer-1 out width
TMW = 640                  # bf16 row: 512 msg | 32 (16 fp32 scalars) | 96 pad
T2W = 64                   # layer-2 expanded row width (fp32)
NCORES = 8
NPC = N // NCORES
WIN = 128
NB = 10
NPAD = NB * WIN            # 1280
NWIN = NCORES * NB         # 80
NBLK = (N + 127) // 128    # 79
MAXI = 1024
WSPLIT = 5                 # windows in first collective slice

FP = mybir.dt.float32
BF = mybir.dt.bfloat16
I16 = mybir.dt.int16

_CACHE = {}

LAST_EXEC_NS = None
LAST_RESULTS = None


def _wrap_idx(vals):
    """int16 gather index layout: idx i -> [i%16, i//16], tiled to 128 partitions."""
    n = vals.shape[0]
    w = np.zeros((16, n // 16), np.int16)
    w[np.arange(n) % 16, np.arange(n) // 16] = vals.astype(np.int16)
    return np.tile(w, (8, 1))


def _build_program(KC, NCH):
    K = KC * NCH
    SL = KC * 128

    nc = bacc.Bacc("TRN2", target_bir_lowering=False, debug=False, num_devices=NCORES)

    # ---- DRAM parameters ----
    xTb_d = nc.dram_tensor("xTb", [IN, N], BF, kind="ExternalInput")
    W1sd_d = nc.dram_tensor("W1sd", [IN, 528], BF, kind="ExternalInput")
    W2_d = nc.dram_tensor("W2r", [128, 4, 8], BF, kind="ExternalInput")
    b2_d = nc.dram_tensor("b2rep", [128, 8], FP, kind="ExternalInput")
    b1_d = nc.dram_tensor("b1rep", [128, C1], FP, kind="ExternalInput")
    ident_d = nc.dram_tensor("identb", [128, 128], BF, kind="ExternalInput")
    ewc8_d = nc.dram_tensor("ewc8", [128, NB, NCH, KC, 8], FP, kind="ExternalInput")
    ew_d = nc.dram_tensor("ew", [128, NB, NCH, KC], FP, kind="ExternalInput")
    srcg_d = nc.dram_tensor("srcg", [128, NB, NCH, KC * 8], I16, kind="ExternalInput")
    dstg_d = nc.dram_tensor("dstg", [128, NB, NCH, KC * 8], I16, kind="ExternalInput")
    srcg2_d = nc.dram_tensor("srcg2", [128, NB, NCH, KC * 8], I16, kind="ExternalInput")
    dstg2_d = nc.dram_tensor("dstg2", [128, NB, NCH, KC * 8], I16, kind="ExternalInput")
    selT_d = nc.dram_tensor("selT", [128, NB, NCH, SL], BF, kind="ExternalInput")

    out_d = nc.dram_tensor("out_own", [NPAD, 4], FP, kind="ExternalOutput")

    # ---- internal DRAM ----
    tableM = nc.dram_tensor("tableM", [NBLK * 128, TMW], BF)
    table2x = nc.dram_tensor("table2x", [NPAD, T2W], FP)
    table2c = nc.dram_tensor("table2c", [NPAD, 8], FP)
    t2cfA = nc.dram_tensor("t2cfA", [NCORES * WSPLIT * 128, 8], FP, addr_space="Shared")
    t2cfB = nc.dram_tensor("t2cfB", [NCORES * (NB - WSPLIT) * 128, 8], FP,
                           addr_space="Shared")
    table2f = nc.dram_tensor("table2f", [NCORES * NPAD, T2W], FP)

    c2_host = _build_program.c2_host
    XCH = 10

    with tile.TileContext(nc) as tc:
        with (
            tc.tile_pool(name="const", bufs=1) as constp,
            tc.tile_pool(name="idx", bufs=1) as idxp,
            tc.tile_pool(name="h1p", bufs=1) as h1p,
            tc.tile_pool(name="selp", bufs=1) as selp,
        ):
            W1sd = constp.tile([IN, 528], BF)
            nc.scalar.dma_start(W1sd[:], W1sd_d[:])
            W2sb = constp.tile([128, 4, 8], BF)
            nc.scalar.dma_start(W2sb[:], W2_d[:])
            b2rep = constp.tile([128, 8], FP)
            nc.scalar.dma_start(b2rep[:], b2_d[:])
            if _build_program.use_b1:
                b1rep = constp.tile([128, C1], FP)
                nc.scalar.dma_start(b1rep[:], b1_d[:])
            identb = constp.tile([128, 128], BF)
            nc.scalar.dma_start(identb[:], ident_d[:])
            ewc8sb = constp.tile([128, NB, NCH, KC, 8], FP)
            ewsb = constp.tile([128, NB, NCH, KC], FP)
            srcg = idxp.tile([128, NB, NCH, KC * 8], I16)
            dstg = idxp.tile([128, NB, NCH, KC * 8], I16)
            srcg2 = idxp.tile([128, NB, NCH, KC * 8], I16)
            dstg2 = idxp.tile([128, NB, NCH, KC * 8], I16)
            selsb = selp.tile([128, NB, NCH, SL], BF)
            h1own = h1p.tile([128, NB, C1], BF)

            # ========== stage 0: packed table via cast-DMA ==========
            with (
                tc.tile_pool(name="s0x", bufs=8) as s0xp,
                tc.tile_pool(name="s0stg", bufs=3) as s0stgp,
                tc.tile_pool(name="s0ps", bufs=3, space="PSUM") as s0ps,
            ):
                xbch = []
                for xc in range((NBLK + XCH - 1) // XCH):
                    c0 = xc * XCH * 128
                    cz = min(N, (xc + 1) * XCH * 128)
                    xb = s0xp.tile([IN, XCH * 128], BF, tag="xb")
                    nc.sync.dma_start(xb[:, : cz - c0], xTb_d[:, c0:cz])
                    xbch.append(xb)
                # L1 metadata queues behind x on SP
                nc.sync.dma_start(srcg[:], srcg_d[:])
                nc.sync.dma_start(dstg[:], dstg_d[:])
                nc.sync.dma_start(ewc8sb[:], ewc8_d[:])

                stg = None
                for b in range(NBLK):
                    rows = min(128, N - b * 128)
                    off = (b % XCH) * 128
                    psAB = s0ps.tile([128, 1024], FP, tag="psAB")
                    xsl = xbch[b // XCH][:, off : off + rows]
                    nc.tensor.matmul(psAB[:rows, 0:512], xsl, W1sd[:, 0:512],
                                     start=True, stop=True)
                    nc.tensor.matmul(psAB[:rows, 512:528], xsl, W1sd[:, 512:528],
                                     start=True, stop=True)
                    if b % 4 == 0:
                        stg = s0stgp.tile([128, 4, 528], FP, tag="stg")
                    bi = b % 4
                    if b == NBLK - 1 and rows < 128:
                        nc.vector.memset(stg[:, bi, :], 0.0)
                    if b % 2 == 0:
                        nc.vector.tensor_copy(stg[:rows, bi, :], psAB[:rows, 0:528])
                    else:
                        nc.scalar.copy(stg[:rows, bi, :], psAB[:rows, 0:528])
                    if bi == 3 or b == NBLK - 1:
                        gsz = bi + 1
                        b0 = b - bi
                        outM = AP(tableM[:].tensor, b0 * 128 * TMW,
                                  [(TMW, 128), (128 * TMW, gsz), (1, 512)])
                        nc.gpsimd.dma_start(outM, stg[:, 0:gsz, 0:512])
                        outS = AP(tableM[:].tensor, b0 * 128 * TMW + 512,
                                  [(TMW, 128), (128 * TMW, gsz), (1, 32)]).bitcast(I16)
                        nc.sync.dma_start(outS, stg[:, 0:gsz, 512:528].bitcast(I16))

            # ================= layer 1 edge phase =================
            with (
                tc.tile_pool(name="g1", bufs=3) as g1p,
                tc.tile_pool(name="gd1", bufs=3) as gd1p,
                tc.tile_pool(name="al1", bufs=3) as al1p,
                tc.tile_pool(name="wend", bufs=2) as wendp,
                tc.tile_pool(name="l2h", bufs=3) as l2hp,
                tc.tile_pool(name="gd2", bufs=1) as gd2p,
                tc.tile_pool(name="a2p", bufs=1) as a2pp,
                tc.tile_pool(name="ps1", bufs=2, space="PSUM") as ps1p,
                tc.tile_pool(name="l2ps", bufs=2, space="PSUM") as l2ps,
                tc.tile_pool(name="l2tp", bufs=2, space="PSUM") as l2tp,
            ):
                # remaining L2 metadata on SP early in L1
                nc.sync.dma_start(srcg2[:], srcg2_d[:])
                nc.sync.dma_start(dstg2[:], dstg2_d[:])
                nc.sync.dma_start(ewsb[:], ew_d[:])

                gd2all = gd2p.tile([128, NB, NCH, KC, T2W], FP)
                a2pall = a2pp.tile([128, NB, NCH, KC, 1], FP)

                for w in range(NB):
                    # per-window sel load (stays resident for layer 2)
                    nc.sync.dma_start(selsb[:, w], selT_d[:, w])
                    psU = ps1p.tile([128, 512], FP)
                    psD = ps1p.tile([128, 8], FP)
                    for ch in range(NCH):
                        g = g1p.tile([128, KC, TMW], BF)
                        nc.gpsimd.dma_gather(
                            g[:], tableM[:], srcg[:, w, ch, :], SL, SL, TMW
                        )
                        gd = gd1p.tile([128, KC, 128], BF)
                        nc.gpsimd.dma_gather(
                            gd[:], tableM[:, 512:640], dstg[:, w, ch, :], SL, SL, 128,
                            elem_step=TMW,
                        )
                        gf = g[:, :, 512:544].bitcast(FP)    # [128, KC, 16]
                        gdf = gd[:, :, 0:64].bitcast(FP)     # [128, KC, 32]
                        a = al1p.tile([128, KC, 8], FP)
                        # alpha adds on gpsimd (same speed as DVE at this size)
                        nc.gpsimd.tensor_tensor(
                            out=a[:], in0=gf[:, :, 0:8], in1=gdf[:, :, 8:16],
                            op=mybir.AluOpType.add,
                        )
                        nc.gpsimd.tensor_tensor(
                            out=a[:], in0=a[:], in1=ewc8sb[:, w, ch],
                            op=mybir.AluOpType.add,
                        )
                        nc.vector.scalar_tensor_tensor(
                            out=a[:], in0=a[:], scalar=0.2, in1=a[:],
                            op0=mybir.AluOpType.mult, op1=mybir.AluOpType.max)
                        ahb = al1p.tile([128, KC, 1, 8], BF)
                        nc.scalar.activation(ahb[:, :, 0, :], a[:],
                                             mybir.ActivationFunctionType.Exp)
                        msg4 = g[:, :, 0:512].rearrange("p t (c h) -> p t c h", h=8)
                        ah4 = ahb[:].to_broadcast([128, KC, 64, 8])
                        nc.vector.tensor_tensor(out=msg4, in0=msg4, in1=ah4,
                                                op=mybir.AluOpType.mult)
                        for t in range(KC):
                            ti = ch * KC + t
                            st = ti == 0
                            sp = ti == K - 1
                            sel = selsb[:, w, ch, t * 128 : (t + 1) * 128]
                            nc.tensor.matmul(psU[:], sel, g[:, t, 0:512],
                                             start=st, stop=sp)
                            nc.tensor.matmul(psD[:], sel, ahb[:, t, 0, :],
                                             start=st, stop=sp)
                    # ---- window finalize: h1 = elu(U/D) in bf16 ----
                    dpe = wendp.tile([128, 8], FP)
                    nc.vector.tensor_scalar_add(dpe[:], psD[:], 1e-16)
                    dr = wendp.tile([128, 1, 8], FP)
                    nc.vector.reciprocal(dr[:, 0, :], dpe[:])
                    h1v = h1own[:, w, :]
                    h1v3 = h1v.rearrange("p (c h) -> p c h", h=8)
                    psU3 = psU[:].rearrange("p (c h) -> p c h", h=8)
                    nc.vector.tensor_tensor(out=h1v3, in0=psU3,
                                            in1=dr[:].to_broadcast([128, 64, 8]),
                                            op=mybir.AluOpType.mult)
                    if _build_program.use_b1:
                        nc.vector.tensor_tensor(out=h1v, in0=h1v, in1=b1rep[:],
                                                op=mybir.AluOpType.add)
                    negp = wendp.tile([128, C1], BF)
                    nc.scalar.activation(negp[:], h1v,
                                         mybir.ActivationFunctionType.Relu,
                                         scale=-1.0)
                    emin = wendp.tile([128, C1], BF)
                    nc.scalar.activation(emin[:], negp[:],
                                         mybir.ActivationFunctionType.Exp,
                                         scale=-1.0)
                    posp = wendp.tile([128, C1], BF)
                    nc.scalar.activation(posp[:], h1v,
                                         mybir.ActivationFunctionType.Relu)
                    nc.vector.scalar_tensor_tensor(
                        out=h1v, in0=posp[:], scalar=-1.0, in1=emin[:],
                        op0=mybir.AluOpType.add, op1=mybir.AluOpType.add,
                    )
                    # ---- layer-2 projection (bf16 transpose path) ----
                    ps2 = l2ps.tile([128, 8], FP)
                    for kc in range(4):
                        tps = l2tp.tile([128, 128], BF)
                        nc.tensor.transpose(
                            tps[:], h1own[:, w, kc * 128 : (kc + 1) * 128], identb[:])
                        tsb = l2hp.tile([128, 128], BF)
                        if kc % 2 == 0:
                            nc.vector.tensor_copy(tsb[:], tps[:])
                        else:
                            nc.scalar.copy(tsb[:], tps[:])
                        nc.tensor.matmul(ps2[:], tsb[:], W2sb[:, kc, :],
                                         start=(kc == 0), stop=(kc == 3))
                    st2 = l2hp.tile([128, 8], FP)
                    nc.scalar.copy(st2[:], ps2[:])
                    nc.sync.dma_start(table2c[w * 128 : (w + 1) * 128, :], st2[:])
                    out2x = AP(table2x[:].tensor, w * 128 * T2W,
                               [(T2W, 128), (1, 8)])
                    nc.sync.dma_start(out2x, st2[:])
                    if w == WSPLIT - 1:
                        nc.gpsimd.collective_compute(
                            "AllGather", mybir.AluOpType.bypass,
                            replica_groups=[list(range(NCORES))],
                            ins=[table2c[0 : WSPLIT * 128, :]],
                            outs=[t2cfA[:]],
                        )

                # dst-side layer-2 gathers + alpha partials (shadow of AllGather B)
                for w in range(NB):
                    for ch in range(NCH):
                        nc.gpsimd.dma_gather(
                            gd2all[:, w, ch], table2x[:], dstg2[:, w, ch, :],
                            SL, SL, T2W,
                        )
                nc.gpsimd.collective_compute(
                    "AllGather", mybir.AluOpType.bypass,
                    replica_groups=[list(range(NCORES))],
                    ins=[table2c[WSPLIT * 128 :, :]],
                    outs=[t2cfB[:]],
                )
                for w in range(NB):
                    for ch in range(NCH):
                        ew_b = ewsb[:, w, ch, :].to_broadcast([128, KC, 1])
                        nc.vector.scalar_tensor_tensor(
                            out=a2pall[:, w, ch], in0=ew_b, scalar=float(c2_host),
                            in1=gd2all[:, w, ch, :, 5:6],
                            op0=mybir.AluOpType.mult, op1=mybir.AluOpType.add,
                        )
                # expansion: drop AG slices into cols 0:8 of table2f
                expA = AP(table2f[:].tensor, 0,
                          [(T2W, WSPLIT * 128), (NPAD * T2W, NCORES), (1, 8)])
                inA = AP(t2cfA[:].tensor, 0,
                         [(8, WSPLIT * 128), (WSPLIT * 128 * 8, NCORES), (1, 8)])
                nc.sync.dma_start(expA, inA)
                expB = AP(table2f[:].tensor, WSPLIT * 128 * T2W,
                          [(T2W, (NB - WSPLIT) * 128), (NPAD * T2W, NCORES), (1, 8)])
                inB = AP(t2cfB[:].tensor, 0,
                         [(8, (NB - WSPLIT) * 128),
                          ((NB - WSPLIT) * 128 * 8, NCORES), (1, 8)])
                nc.sync.dma_start(expB, inB)

            # ================= layer 2 edge phase =================
            with (
                tc.tile_pool(name="g2", bufs=3) as g2p,
                tc.tile_pool(name="al2", bufs=3) as al2p,
                tc.tile_pool(name="wend2", bufs=2) as wend2p,
                tc.tile_pool(name="ps2p", bufs=2, space="PSUM") as ps2pp,
            ):
                for w in range(NB):
                    psO = ps2pp.tile([128, 8], FP)
                    for ch in range(NCH):
                        gs = g2p.tile([128, KC, T2W], FP)
                        nc.gpsimd.dma_gather(
                            gs[:], table2f[:], srcg2[:, w, ch, :], SL, SL, T2W
                        )
                        a2 = al2p.tile([128, KC, 1], FP)
                        nc.vector.tensor_tensor(out=a2[:], in0=gs[:, :, 4:5],
                                                in1=a2pall[:, w, ch],
                                                op=mybir.AluOpType.add)
                        nc.vector.scalar_tensor_tensor(
                            out=a2[:], in0=a2[:], scalar=0.2, in1=a2[:],
                            op0=mybir.AluOpType.mult, op1=mybir.AluOpType.max)
                        nc.scalar.activation(gs[:, :, 4:5], a2[:],
                                             mybir.ActivationFunctionType.Exp)
                        ah = gs[:, :, 4:5].to_broadcast([128, KC, 4])
                        nc.vector.tensor_tensor(out=gs[:, :, 0:4], in0=gs[:, :, 0:4],
                                                in1=ah, op=mybir.AluOpType.mult)
                        g5b = al2p.tile([128, KC, 8], BF)
                        nc.vector.tensor_copy(g5b[:, :, 0:5], gs[:, :, 0:5])
                        for t in range(KC):
                            ti = ch * KC + t
                            sel = selsb[:, w, ch, t * 128 : (t + 1) * 128]
                            nc.tensor.matmul(psO[:, 0:5], sel, g5b[:, t, 0:5],
                                             start=(ti == 0), stop=(ti == K - 1))
                    dpe = wend2p.tile([128, 1], FP)
                    nc.vector.tensor_scalar_add(dpe[:], psO[:, 4:5], 1e-16)
                    dr = wend2p.tile([128, 1], FP)
                    nc.vector.reciprocal(dr[:], dpe[:])
                    ob = wend2p.tile([128, 8], FP)
                    nc.vector.tensor_tensor(out=ob[:, 0:4], in0=psO[:, 0:4],
                                            in1=dr[:].to_broadcast([128, 4]),
                                            op=mybir.AluOpType.mult)
                    if _build_program.use_b2:
                        nc.vector.tensor_tensor(out=ob[:, 0:4], in0=ob[:, 0:4],
                                                in1=b2rep[:, 0:4],
                                                op=mybir.AluOpType.add)
                    nc.sync.dma_start(out_d[w * 128 : (w + 1) * 128, :], ob[:, 0:4])

    nc.compile()
    return nc


def _balance_windows(dst):
    """Greedy in-degree balancing of nodes into NWIN windows of WIN slots."""
    import heapq

    indeg = np.bincount(dst, minlength=N)
    order = np.argsort(-indeg, kind="stable")
    heap = [(0, w) for w in range(NWIN)]
    heapq.heapify(heap)
    fill = np.zeros(NWIN, np.int64)
    node_win = np.zeros(N, np.int64)
    node_slot = np.zeros(N, np.int64)
    for n in order:
        cnt, w = heapq.heappop(heap)
        node_win[n] = w
        node_slot[n] = fill[w]
        fill[w] += 1
        if fill[w] < WIN:
            heapq.heappush(heap, (cnt + int(indeg[n]), w))
    return node_win, node_slot


def _prepare(x, edge_index, edge_weight, W1, att_src1, att_dst1, att_edge1, We1, b1,
             W2, att_src2, att_dst2, att_edge2, We2, b2):
    x = np.asarray(x, np.float32)
    ei = np.asarray(edge_index)
    ew = np.asarray(edge_weight, np.float32)
    W1 = np.asarray(W1, np.float32)
    att_src1 = np.asarray(att_src1, np.float32)
    att_dst1 = np.asarray(att_dst1, np.float32)
    att_edge1 = np.asarray(att_edge1, np.float32)
    We1 = np.asarray(We1, np.float32)
    b1 = np.asarray(b1, np.float32)
    W2 = np.asarray(W2, np.float32)
    att_src2 = np.asarray(att_src2, np.float32)
    att_dst2 = np.asarray(att_dst2, np.float32)
    att_edge2 = np.asarray(att_edge2, np.float32)
    We2 = np.asarray(We2, np.float32)
    b2 = np.asarray(b2, np.float32)

    # ---------- weight folding ----------
    W1r = W1.reshape(IN, H, HID)
    Wsrc = np.einsum("khc,hc->kh", W1r, att_src1)
    Wdst = np.einsum("khc,hc->kh", W1r, att_dst1)
    c1 = (We1.reshape(H, HID) * att_edge1).sum(1).astype(np.float32)  # [H]

    # head-minor column order: new col c*8+h = old h*64+c
    cols = np.tile(np.arange(H), HID) * HID + np.repeat(np.arange(HID), H)
    W1p = np.ascontiguousarray(W1[:, cols])
    b1p = b1[cols]
    W2p = W2[cols, :]
    W1sd = np.concatenate([W1p, Wsrc, Wdst], axis=1)  # [IN, 528]

    Waug2 = np.zeros((C1, 8), np.float32)
    Waug2[:, 0:4] = W2p
    Waug2[:, 4] = W2p @ att_src2[0]
    Waug2[:, 5] = W2p @ att_dst2[0]
    W2resh = np.ascontiguousarray(Waug2.reshape(4, 128, 8).transpose(1, 0, 2))
    c2 = float((We2[0] * att_edge2[0]).sum())
    _build_program.c2_host = c2
    _build_program.use_b1 = bool(np.any(b1))
    _build_program.use_b2 = bool(np.any(b2))

    # ---------- edge partitioning ----------
    src = np.asarray(ei[0], np.int64)
    dst = np.asarray(ei[1], np.int64)

    node_win, node_slot = _balance_windows(dst)
    node_core = node_win // NB
    node_w = node_win % NB
    node_local = node_w * WIN + node_slot
    node_gpad = node_core * NPAD + node_local

    ekey = node_win[dst]
    order = np.argsort(ekey, kind="stable")
    s_s, d_s, w_s = src[order], dst[order], ew[order]
    core_of = node_core[d_s]
    win_of = node_w[d_s]
    loc_of = node_slot[d_s]

    cnt = np.bincount(node_win[d_s], minlength=NWIN)
    K = int(np.ceil(cnt.max() / 128.0))
    NCHo = os.environ.get("BASS_GAT_NCH")
    if NCHo is not None:
        NCH = int(NCHo)
        KC = (K + NCH - 1) // NCH
    else:
        NCH = 2
        while ((K + NCH - 1) // NCH) * 128 > MAXI:
            NCH += 1
        KC = (K + NCH - 1) // NCH
    K = KC * NCH
    SL = KC * 128
    SW = K * 128

    in_maps = []
    base_rep = {
        "xTb": np.ascontiguousarray(x.T).astype(ml_dtypes.bfloat16),
        "W1sd": W1sd.astype(ml_dtypes.bfloat16),
        "W2r": W2resh.astype(ml_dtypes.bfloat16),
        "b2rep": np.tile(np.concatenate([b2, np.zeros(4, np.float32)])[None, :],
                         (128, 1)),
        "b1rep": np.tile(b1p[None, :], (128, 1)),
        "identb": np.eye(128, dtype=np.float32).astype(ml_dtypes.bfloat16),
    }

    for c in range(NCORES):
        m = dict(base_rep)
        srcgm = np.zeros((NB, NCH, 128, KC * 8), np.int16)
        dstgm = np.zeros((NB, NCH, 128, KC * 8), np.int16)
        srcg2m = np.zeros((NB, NCH, 128, KC * 8), np.int16)
        dstg2m = np.zeros((NB, NCH, 128, KC * 8), np.int16)
        ews = np.zeros((NB, NCH, KC, 128), np.float32)
        selTm = np.zeros((NB, NCH, 128, SL), np.float32)
        sel_c = core_of == c
        for w in range(NB):
            es = np.nonzero(sel_c & (win_of == w))[0]
            ns = len(es)
            ssrc = np.zeros(SW, np.int64)
            sdst = np.zeros(SW, np.int64)
            sew = np.zeros(SW, np.float32)
            sloc = np.full(SW, -1, np.int64)
            ssrc[:ns] = s_s[es]
            sdst[:ns] = d_s[es]
            sew[:ns] = w_s[es]
            sloc[:ns] = loc_of[es]
            for ch in range(NCH):
                sl = slice(ch * SL, (ch + 1) * SL)
                srcgm[w, ch] = _wrap_idx(ssrc[sl])
                dstgm[w, ch] = _wrap_idx(sdst[sl])
                srcg2m[w, ch] = _wrap_idx(node_gpad[ssrc[sl]])
                dstg2m[w, ch] = _wrap_idx(node_local[sdst[sl]])
                ews[w, ch] = sew[sl].reshape(KC, 128)
                lc = sloc[sl]
                valid = np.nonzero(lc >= 0)[0]
                tt, pp = valid // 128, valid % 128
                selTm[w, ch, pp, tt * 128 + lc[valid]] = 1.0
        m["srcg"] = np.ascontiguousarray(srcgm.transpose(2, 0, 1, 3))
        m["dstg"] = np.ascontiguousarray(dstgm.transpose(2, 0, 1, 3))
        m["srcg2"] = np.ascontiguousarray(srcg2m.transpose(2, 0, 1, 3))
        m["dstg2"] = np.ascontiguousarray(dstg2m.transpose(2, 0, 1, 3))
        ewt = np.ascontiguousarray(ews.transpose(3, 0, 1, 2))
        m["ew"] = ewt
        m["ewc8"] = np.ascontiguousarray(ewt[..., None] * c1)
        m["selT"] = np.ascontiguousarray(
            selTm.transpose(2, 0, 1, 3)).astype(ml_dtypes.bfloat16)
        in_maps.append(m)

    meta = (node_core, node_local)
    return in_maps, KC, NCH, c2, meta


def kernel(**inputs):
    global LAST_EXEC_NS, LAST_RESULTS
    in_maps, KC, NCH, c2, meta = _prepare(**inputs)
    key = (KC, NCH, c2, _build_program.use_b1, _build_program.use_b2)
    if key not in _CACHE:
        _CACHE[key] = _build_program(KC, NCH)
    nc = _CACHE[key]

    trace = os.environ.get("BASS_GAT_TRACE", "0") == "1"
    res = run_bass_kernel_spmd(nc, in_maps, list(range(NCORES)), trace=trace)
    LAST_EXEC_NS = res.exec_time_ns
    LAST_RESULTS = res
    node_core, node_local = meta
    per_core = [res.results[c]["out_own"] for c in range(NCORES)]
    out = np.empty((N, 4), np.float32)
    for c in range(NCORES):
        mask = node_core == c
        out[mask] = per_core[c][node_local[mask]]
    return out


# revision 9
# speedup vs baseline: 1.1369x; 1.1369x over previous
"""Trainium2 Bass kernel for a 2-layer GAT (nn_GAT_781684048444).

Strategy (8 NeuronCores, SPMD) — v2:
  - Nodes assigned to 80 windows (8 cores x 10 windows x 128 slots) by greedy
    in-degree balancing; edges grouped by dst window, padded to K 128-edge
    tiles; one static program serves all cores. Output inverse-permuted.
  - Stage 0 (replicated): ONE bf16 matmul per 128-node block computes the
    packed row [512 bf16 msg (head-minor) | 8 fp32 s_src | 8 fp32 s_dst]
    (attention dots folded into the weight matrix; the fp32 PSUM result is
    staged to SBUF once, msg cols reach DRAM through a gpsimd cast-DMA
    (fp32->bf16 in the DMA), scalar cols through a raw bit-copy DMA on SP).
    No separate fp32 x load, no second matmul, no big DVE copy traffic.
  - Layer-1 edge phase: gather 1280B rows by src + 256B scalar blocks by dst;
    alpha adds run on gpsimd, leaky-relu on DVE, exp on ACT; the bf16
    msg *= alpha broadcast runs in DVE 2x mode; scatter-add via resident
    one-hot selT bf16 matmuls into PSUM. h1 = elu(U/D) with the elu expressed
    as relu/exp on ACT (elu(x) = relu(x) + exp(-relu(-x)) - 1), only the
    final combine on DVE. h1 kept bf16.
  - Layer-2 projection per window (bf16 transpose + matmul) feeds a compact
    [NPAD, 8] fp32 table; the AllGather is SPLIT in two (windows 0-4 issued
    mid-layer-1, 5-9 at the end) so most of its fixed cost overlaps layer 1.
    dst-side gathers + alpha partials are computed in the shadow of the
    second collective; src gathers + the final gather-attend-scatter follow.
"""

import os
import sys

import ml_dtypes
import numpy as np

sys.path.insert(0, "/opt/trn_rl_repo")

from concourse import bacc, bass, mybir, tile  # noqa: E402
from concourse.bass import AP  # noqa: E402
from concourse.bass_utils import run_bass_kernel_spmd  # noqa: E402

N, E = 10000, 160000
IN, HID, OUT, H = 128, 64, 4, 8
C1 = H * HID               # 512 layer-1 out width
TMW = 320                  # fp32 row: 256 (512 bf16 msg) | 8 ssrc | 8 sdst | 48 pad
T2W = 64                   # layer-2 expanded row width (fp32)
NCORES = 8
NPC = N // NCORES
WIN = 128
NB = 10
NPAD = NB * WIN            # 1280
NWIN = NCORES * NB         # 80
NBLK = (N + 127) // 128    # 79
MAXI = 1024
WSPLIT = 5                 # windows in first collective slice

FP = mybir.dt.float32
BF = mybir.dt.bfloat16
I16 = mybir.dt.int16

_CACHE = {}

LAST_EXEC_NS = None
LAST_RESULTS = None


def _wrap_idx(vals):
    """int16 gather index layout: idx i -> [i%16, i//16], tiled to 128 partitions."""
    n = vals.shape[0]
    w = np.zeros((16, n // 16), np.int16)
    w[np.arange(n) % 16, np.arange(n) // 16] = vals.astype(np.int16)
    return np.tile(w, (8, 1))


def _build_program(KC, NCH):
    K = KC * NCH
    SL = KC * 128

    nc = bacc.Bacc("TRN2", target_bir_lowering=False, debug=False, num_devices=NCORES)

    # ---- DRAM parameters ----
    xTb_d = nc.dram_tensor("xTb", [IN, N], BF, kind="ExternalInput")
    W1sd_d = nc.dram_tensor("W1sd", [IN, 528], BF, kind="ExternalInput")
    W2_d = nc.dram_tensor("W2r", [128, 4, 8], BF, kind="ExternalInput")
    b2_d = nc.dram_tensor("b2rep", [128, 8], FP, kind="ExternalInput")
    b1_d = nc.dram_tensor("b1rep", [128, C1], FP, kind="ExternalInput")
    ident_d = nc.dram_tensor("identb", [128, 128], BF, kind="ExternalInput")
    ewc8_d = nc.dram_tensor("ewc8", [128, NB, NCH, KC, 8], FP, kind="ExternalInput")
    ew_d = nc.dram_tensor("ew", [128, NB, NCH, KC], FP, kind="ExternalInput")
    srcg_d = nc.dram_tensor("srcg", [128, NB, NCH, KC * 8], I16, kind="ExternalInput")
    dstg_d = nc.dram_tensor("dstg", [128, NB, NCH, KC * 8], I16, kind="ExternalInput")
    srcg2_d = nc.dram_tensor("srcg2", [128, NB, NCH, KC * 8], I16, kind="ExternalInput")
    dstg2_d = nc.dram_tensor("dstg2", [128, NB, NCH, KC * 8], I16, kind="ExternalInput")
    selT_d = nc.dram_tensor("selT", [128, NB, NCH, SL], BF, kind="ExternalInput")

    out_d = nc.dram_tensor("out_own", [NPAD, 4], FP, kind="ExternalOutput")

    # ---- internal DRAM ----
    tableM = nc.dram_tensor("tableM", [NBLK * 128, TMW], FP)
    table2x = nc.dram_tensor("table2x", [NPAD, T2W], FP)
    table2c = nc.dram_tensor("table2c", [NPAD, 8], FP)
    t2cfA = nc.dram_tensor("t2cfA", [NCORES * WSPLIT * 128, 8], FP, addr_space="Shared")
    t2cfB = nc.dram_tensor("t2cfB", [NCORES * (NB - WSPLIT) * 128, 8], FP,
                           addr_space="Shared")
    table2f = nc.dram_tensor("table2f", [NCORES * NPAD, T2W], FP)

    c2_host = _build_program.c2_host
    XCH = 10

    with tile.TileContext(nc) as tc:
        with (
            tc.tile_pool(name="const", bufs=1) as constp,
            tc.tile_pool(name="idx", bufs=1) as idxp,
            tc.tile_pool(name="h1p", bufs=1) as h1p,
            tc.tile_pool(name="selp", bufs=1) as selp,
        ):
            W1sd = constp.tile([IN, 528], BF)
            nc.scalar.dma_start(W1sd[:], W1sd_d[:])
            W2sb = constp.tile([128, 4, 8], BF)
            nc.scalar.dma_start(W2sb[:], W2_d[:])
            b2rep = constp.tile([128, 8], FP)
            nc.scalar.dma_start(b2rep[:], b2_d[:])
            if _build_program.use_b1:
                b1rep = constp.tile([128, C1], FP)
                nc.scalar.dma_start(b1rep[:], b1_d[:])
            identb = constp.tile([128, 128], BF)
            nc.scalar.dma_start(identb[:], ident_d[:])
            ewc8sb = constp.tile([128, NB, NCH, KC, 8], FP)
            ewsb = constp.tile([128, NB, NCH, KC], FP)
            srcg = idxp.tile([128, NB, NCH, KC * 8], I16)
            dstg = idxp.tile([128, NB, NCH, KC * 8], I16)
            srcg2 = idxp.tile([128, NB, NCH, KC * 8], I16)
            dstg2 = idxp.tile([128, NB, NCH, KC * 8], I16)
            selsb = selp.tile([128, NB, NCH, SL], BF)
            h1own = h1p.tile([128, NB, C1], BF)

            # ========== stage 0: packed table via cast-DMA ==========
            with (
                tc.tile_pool(name="s0x", bufs=8) as s0xp,
                tc.tile_pool(name="s0stg", bufs=3) as s0stgp,
                tc.tile_pool(name="s0ps", bufs=3, space="PSUM") as s0ps,
            ):
                xbch = []
                for xc in range((NBLK + XCH - 1) // XCH):
                    c0 = xc * XCH * 128
                    cz = min(N, (xc + 1) * XCH * 128)
                    xb = s0xp.tile([IN, XCH * 128], BF, tag="xb")
                    nc.sync.dma_start(xb[:, : cz - c0], xTb_d[:, c0:cz])
                    xbch.append(xb)
                # L1 metadata queues behind x on SP
                nc.sync.dma_start(srcg[:], srcg_d[:])
                nc.sync.dma_start(dstg[:], dstg_d[:])
                nc.sync.dma_start(ewc8sb[:], ewc8_d[:])

                stg = None
                for b in range(NBLK):
                    rows = min(128, N - b * 128)
                    off = (b % XCH) * 128
                    psAB = s0ps.tile([128, 1024], FP, tag="psAB")
                    xsl = xbch[b // XCH][:, off : off + rows]
                    nc.tensor.matmul(psAB[:rows, 0:512], xsl, W1sd[:, 0:512],
                                     start=True, stop=True)
                    nc.tensor.matmul(psAB[:rows, 512:528], xsl, W1sd[:, 512:528],
                                     start=True, stop=True)
                    if b % 4 == 0:
                        stg = s0stgp.tile([128, 4, 528], FP, tag="stg")
                    bi = b % 4
                    if b == NBLK - 1 and rows < 128:
                        nc.vector.memset(stg[:, bi, :], 0.0)
                    if b % 2 == 0:
                        nc.vector.tensor_copy(stg[:rows, bi, :], psAB[:rows, 0:528])
                    else:
                        nc.scalar.copy(stg[:rows, bi, :], psAB[:rows, 0:528])
                    if bi == 3 or b == NBLK - 1:
                        gsz = bi + 1
                        b0 = b - bi
                        outM = AP(tableM[:].tensor, b0 * 128 * TMW,
                                  [(TMW, 128), (128 * TMW, gsz), (1, 256)]).bitcast(BF)
                        nc.gpsimd.dma_start(outM, stg[:, 0:gsz, 0:512])
                        outS = AP(tableM[:].tensor, b0 * 128 * TMW + 256,
                                  [(TMW, 128), (128 * TMW, gsz), (1, 16)])
                        nc.sync.dma_start(outS, stg[:, 0:gsz, 512:528])

            # ================= layer 1 edge phase =================
            with (
                tc.tile_pool(name="g1", bufs=3) as g1p,
                tc.tile_pool(name="gd1", bufs=3) as gd1p,
                tc.tile_pool(name="al1", bufs=3) as al1p,
                tc.tile_pool(name="wend", bufs=2) as wendp,
                tc.tile_pool(name="l2h", bufs=3) as l2hp,
                tc.tile_pool(name="gd2", bufs=1) as gd2p,
                tc.tile_pool(name="a2p", bufs=1) as a2pp,
                tc.tile_pool(name="ps1", bufs=2, space="PSUM") as ps1p,
                tc.tile_pool(name="l2ps", bufs=2, space="PSUM") as l2ps,
                tc.tile_pool(name="l2tp", bufs=2, space="PSUM") as l2tp,
            ):
                # remaining L2 metadata on SP early in L1
                nc.sync.dma_start(srcg2[:], srcg2_d[:])
                nc.sync.dma_start(dstg2[:], dstg2_d[:])
                nc.sync.dma_start(ewsb[:], ew_d[:])

                gd2all = gd2p.tile([128, NB, NCH, KC, T2W], FP)
                a2pall = a2pp.tile([128, NB, NCH, KC, 1], FP)

                for w in range(NB):
                    # per-window sel load (stays resident for layer 2)
                    nc.sync.dma_start(selsb[:, w], selT_d[:, w])
                    psU = ps1p.tile([128, 512], FP)
                    psD = ps1p.tile([128, 8], FP)
                    for ch in range(NCH):
                        g = g1p.tile([128, KC, TMW], FP)
                        nc.gpsimd.dma_gather(
                            g[:], tableM[:], srcg[:, w, ch, :], SL, SL, TMW
                        )
                        gd = gd1p.tile([128, KC, 64], FP)
                        nc.gpsimd.dma_gather(
                            gd[:], tableM[:, 256:320], dstg[:, w, ch, :], SL, SL, 64,
                            elem_step=TMW,
                        )
                        a = al1p.tile([128, KC, 8], FP)
                        # alpha adds on gpsimd (same speed as DVE at this size)
                        nc.gpsimd.tensor_tensor(
                            out=a[:], in0=g[:, :, 256:264], in1=gd[:, :, 8:16],
                            op=mybir.AluOpType.add,
                        )
                        nc.gpsimd.tensor_tensor(
                            out=a[:], in0=a[:], in1=ewc8sb[:, w, ch],
                            op=mybir.AluOpType.add,
                        )
                        nc.vector.scalar_tensor_tensor(
                            out=a[:], in0=a[:], scalar=0.2, in1=a[:],
                            op0=mybir.AluOpType.mult, op1=mybir.AluOpType.max)
                        ahb = al1p.tile([128, KC, 1, 8], BF)
                        nc.scalar.activation(ahb[:, :, 0, :], a[:],
                                             mybir.ActivationFunctionType.Exp)
                        mv = g[:, :, 0:256].bitcast(BF)
                        msg4 = mv.rearrange("p t (c h) -> p t c h", h=8)
                        ah4 = ahb[:].to_broadcast([128, KC, 64, 8])
                        nc.vector.tensor_tensor(out=msg4, in0=msg4, in1=ah4,
                                                op=mybir.AluOpType.mult)
                        for t in range(KC):
                            ti = ch * KC + t
                            st = ti == 0
                            sp = ti == K - 1
                            sel = selsb[:, w, ch, t * 128 : (t + 1) * 128]
                            nc.tensor.matmul(psU[:], sel, g[:, t, 0:256].bitcast(BF),
                                             start=st, stop=sp)
                            nc.tensor.matmul(psD[:], sel, ahb[:, t, 0, :],
                                             start=st, stop=sp)
                    # ---- window finalize: h1 = elu(U/D) in bf16 ----
                    dpe = wendp.tile([128, 8], FP)
                    nc.vector.tensor_scalar_add(dpe[:], psD[:], 1e-16)
                    dr = wendp.tile([128, 1, 8], FP)
                    nc.vector.reciprocal(dr[:, 0, :], dpe[:])
                    h1v = h1own[:, w, :]
                    h1v3 = h1v.rearrange("p (c h) -> p c h", h=8)
                    psU3 = psU[:].rearrange("p (c h) -> p c h", h=8)
                    nc.vector.tensor_tensor(out=h1v3, in0=psU3,
                                            in1=dr[:].to_broadcast([128, 64, 8]),
                                            op=mybir.AluOpType.mult)
                    if _build_program.use_b1:
                        nc.vector.tensor_tensor(out=h1v, in0=h1v, in1=b1rep[:],
                                                op=mybir.AluOpType.add)
                    negp = wendp.tile([128, C1], BF)
                    nc.scalar.activation(negp[:], h1v,
                                         mybir.ActivationFunctionType.Relu,
                                         scale=-1.0)
                    emin = wendp.tile([128, C1], BF)
                    nc.scalar.activation(emin[:], negp[:],
                                         mybir.ActivationFunctionType.Exp,
                                         scale=-1.0)
                    posp = wendp.tile([128, C1], BF)
                    nc.scalar.activation(posp[:], h1v,
                                         mybir.ActivationFunctionType.Relu)
                    nc.vector.scalar_tensor_tensor(
                        out=h1v, in0=posp[:], scalar=-1.0, in1=emin[:],
                        op0=mybir.AluOpType.add, op1=mybir.AluOpType.add,
                    )
                    # ---- layer-2 projection (bf16 transpose path) ----
                    ps2 = l2ps.tile([128, 8], FP)
                    for kc in range(4):
                        tps = l2tp.tile([128, 128], BF)
                        nc.tensor.transpose(
                            tps[:], h1own[:, w, kc * 128 : (kc + 1) * 128], identb[:])
                        tsb = l2hp.tile([128, 128], BF)
                        if kc % 2 == 0:
                            nc.vector.tensor_copy(tsb[:], tps[:])
                        else:
                            nc.scalar.copy(tsb[:], tps[:])
                        nc.tensor.matmul(ps2[:], tsb[:], W2sb[:, kc, :],
                                         start=(kc == 0), stop=(kc == 3))
                    st2 = l2hp.tile([128, 8], FP)
                    nc.scalar.copy(st2[:], ps2[:])
                    nc.sync.dma_start(table2c[w * 128 : (w + 1) * 128, :], st2[:])
                    out2x = AP(table2x[:].tensor, w * 128 * T2W,
                               [(T2W, 128), (1, 8)])
                    nc.sync.dma_start(out2x, st2[:])
                    if w == WSPLIT - 1:
                        nc.gpsimd.collective_compute(
                            "AllGather", mybir.AluOpType.bypass,
                            replica_groups=[list(range(NCORES))],
                            ins=[table2c[0 : WSPLIT * 128, :]],
                            outs=[t2cfA[:]],
                        )

                # dst-side layer-2 gathers + alpha partials (shadow of AllGather B)
                for w in range(NB):
                    for ch in range(NCH):
                        nc.gpsimd.dma_gather(
                            gd2all[:, w, ch], table2x[:], dstg2[:, w, ch, :],
                            SL, SL, T2W,
                        )
                nc.gpsimd.collective_compute(
                    "AllGather", mybir.AluOpType.bypass,
                    replica_groups=[list(range(NCORES))],
                    ins=[table2c[WSPLIT * 128 :, :]],
                    outs=[t2cfB[:]],
                )
                for w in range(NB):
                    for ch in range(NCH):
                        ew_b = ewsb[:, w, ch, :].to_broadcast([128, KC, 1])
                        nc.vector.scalar_tensor_tensor(
                            out=a2pall[:, w, ch], in0=ew_b, scalar=float(c2_host),
                            in1=gd2all[:, w, ch, :, 5:6],
                            op0=mybir.AluOpType.mult, op1=mybir.AluOpType.add,
                        )
                # expansion: drop AG slices into cols 0:8 of table2f
                expA = AP(table2f[:].tensor, 0,
                          [(T2W, WSPLIT * 128), (NPAD * T2W, NCORES), (1, 8)])
                inA = AP(t2cfA[:].tensor, 0,
                         [(8, WSPLIT * 128), (WSPLIT * 128 * 8, NCORES), (1, 8)])
                nc.sync.dma_start(expA, inA)
                expB = AP(table2f[:].tensor, WSPLIT * 128 * T2W,
                          [(T2W, (NB - WSPLIT) * 128), (NPAD * T2W, NCORES), (1, 8)])
                inB = AP(t2cfB[:].tensor, 0,
                         [(8, (NB - WSPLIT) * 128),
                          ((NB - WSPLIT) * 128 * 8, NCORES), (1, 8)])
                nc.sync.dma_start(expB, inB)

            # ================= layer 2 edge phase =================
            with (
                tc.tile_pool(name="g2", bufs=3) as g2p,
                tc.tile_pool(name="al2", bufs=3) as al2p,
                tc.tile_pool(name="wend2", bufs=2) as wend2p,
                tc.tile_pool(name="ps2p", bufs=2, space="PSUM") as ps2pp,
            ):
                for w in range(NB):
                    psO = ps2pp.tile([128, 8], FP)
                    for ch in range(NCH):
                        gs = g2p.tile([128, KC, T2W], FP)
                        nc.gpsimd.dma_gather(
                            gs[:], table2f[:], srcg2[:, w, ch, :], SL, SL, T2W
                        )
                        a2 = al2p.tile([128, KC, 1], FP)
                        nc.vector.tensor_tensor(out=a2[:], in0=gs[:, :, 4:5],
                                                in1=a2pall[:, w, ch],
                                                op=mybir.AluOpType.add)
                        nc.vector.scalar_tensor_tensor(
                            out=a2[:], in0=a2[:], scalar=0.2, in1=a2[:],
                            op0=mybir.AluOpType.mult, op1=mybir.AluOpType.max)
                        nc.scalar.activation(gs[:, :, 4:5], a2[:],
                                             mybir.ActivationFunctionType.Exp)
                        ah = gs[:, :, 4:5].to_broadcast([128, KC, 4])
                        nc.vector.tensor_tensor(out=gs[:, :, 0:4], in0=gs[:, :, 0:4],
                                                in1=ah, op=mybir.AluOpType.mult)
                        g5b = al2p.tile([128, KC, 8], BF)
                        nc.vector.tensor_copy(g5b[:, :, 0:5], gs[:, :, 0:5])
                        for t in range(KC):
                            ti = ch * KC + t
                            sel = selsb[:, w, ch, t * 128 : (t + 1) * 128]
                            nc.tensor.matmul(psO[:, 0:5], sel, g5b[:, t, 0:5],
                                             start=(ti == 0), stop=(ti == K - 1))
                    dpe = wend2p.tile([128, 1], FP)
                    nc.vector.tensor_scalar_add(dpe[:], psO[:, 4:5], 1e-16)
                    dr = wend2p.tile([128, 1], FP)
                    nc.vector.reciprocal(dr[:], dpe[:])
                    ob = wend2p.tile([128, 8], FP)
                    nc.vector.tensor_tensor(out=ob[:, 0:4], in0=psO[:, 0:4],
                                            in1=dr[:].to_broadcast([128, 4]),
                                            op=mybir.AluOpType.mult)
                    if _build_program.use_b2:
                        nc.vector.tensor_tensor(out=ob[:, 0:4], in0=ob[:, 0:4],
                                                in1=b2rep[:, 0:4],
                                                op=mybir.AluOpType.add)
                    nc.sync.dma_start(out_d[w * 128 : (w + 1) * 128, :], ob[:, 0:4])

    nc.compile()
    return nc


def _balance_windows(dst):
    """Greedy in-degree balancing of nodes into NWIN windows of WIN slots."""
    import heapq

    indeg = np.bincount(dst, minlength=N)
    order = np.argsort(-indeg, kind="stable")
    heap = [(0, w) for w in range(NWIN)]
    heapq.heapify(heap)
    fill = np.zeros(NWIN, np.int64)
    node_win = np.zeros(N, np.int64)
    node_slot = np.zeros(N, np.int64)
    for n in order:
        cnt, w = heapq.heappop(heap)
        node_win[n] = w
        node_slot[n] = fill[w]
        fill[w] += 1
        if fill[w] < WIN:
            heapq.heappush(heap, (cnt + int(indeg[n]), w))
    return node_win, node_slot


def _prepare(x, edge_index, edge_weight, W1, att_src1, att_dst1, att_edge1, We1, b1,
             W2, att_src2, att_dst2, att_edge2, We2, b2):
    x = np.asarray(x, np.float32)
    ei = np.asarray(edge_index)
    ew = np.asarray(edge_weight, np.float32)
    W1 = np.asarray(W1, np.float32)
    att_src1 = np.asarray(att_src1, np.float32)
    att_dst1 = np.asarray(att_dst1, np.float32)
    att_edge1 = np.asarray(att_edge1, np.float32)
    We1 = np.asarray(We1, np.float32)
    b1 = np.asarray(b1, np.float32)
    W2 = np.asarray(W2, np.float32)
    att_src2 = np.asarray(att_src2, np.float32)
    att_dst2 = np.asarray(att_dst2, np.float32)
    att_edge2 = np.asarray(att_edge2, np.float32)
    We2 = np.asarray(We2, np.float32)
    b2 = np.asarray(b2, np.float32)

    # ---------- weight folding ----------
    W1r = W1.reshape(IN, H, HID)
    Wsrc = np.einsum("khc,hc->kh", W1r, att_src1)
    Wdst = np.einsum("khc,hc->kh", W1r, att_dst1)
    c1 = (We1.reshape(H, HID) * att_edge1).sum(1).astype(np.float32)  # [H]

    # head-minor column order: new col c*8+h = old h*64+c
    cols = np.tile(np.arange(H), HID) * HID + np.repeat(np.arange(HID), H)
    W1p = np.ascontiguousarray(W1[:, cols])
    b1p = b1[cols]
    W2p = W2[cols, :]
    W1sd = np.concatenate([W1p, Wsrc, Wdst], axis=1)  # [IN, 528]

    Waug2 = np.zeros((C1, 8), np.float32)
    Waug2[:, 0:4] = W2p
    Waug2[:, 4] = W2p @ att_src2[0]
    Waug2[:, 5] = W2p @ att_dst2[0]
    W2resh = np.ascontiguousarray(Waug2.reshape(4, 128, 8).transpose(1, 0, 2))
    c2 = float((We2[0] * att_edge2[0]).sum())
    _build_program.c2_host = c2
    _build_program.use_b1 = bool(np.any(b1))
    _build_program.use_b2 = bool(np.any(b2))

    # ---------- edge partitioning ----------
    src = np.asarray(ei[0], np.int64)
    dst = np.asarray(ei[1], np.int64)

    node_win, node_slot = _balance_windows(dst)
    node_core = node_win // NB
    node_w = node_win % NB
    node_local = node_w * WIN + node_slot
    node_gpad = node_core * NPAD + node_local

    ekey = node_win[dst]
    order = np.argsort(ekey, kind="stable")
    s_s, d_s, w_s = src[order], dst[order], ew[order]
    core_of = node_core[d_s]
    win_of = node_w[d_s]
    loc_of = node_slot[d_s]

    cnt = np.bincount(node_win[d_s], minlength=NWIN)
    K = int(np.ceil(cnt.max() / 128.0))
    NCHo = os.environ.get("BASS_GAT_NCH")
    if NCHo is not None:
        NCH = int(NCHo)
        KC = (K + NCH - 1) // NCH
    else:
        NCH = 2
        while ((K + NCH - 1) // NCH) * 128 > MAXI:
            NCH += 1
        KC = (K + NCH - 1) // NCH
    K = KC * NCH
    SL = KC * 128
    SW = K * 128

    in_maps = []
    base_rep = {
        "xTb": np.ascontiguousarray(x.T).astype(ml_dtypes.bfloat16),
        "W1sd": W1sd.astype(ml_dtypes.bfloat16),
        "W2r": W2resh.astype(ml_dtypes.bfloat16),
        "b2rep": np.tile(np.concatenate([b2, np.zeros(4, np.float32)])[None, :],
                         (128, 1)),
        "b1rep": np.tile(b1p[None, :], (128, 1)),
        "identb": np.eye(128, dtype=np.float32).astype(ml_dtypes.bfloat16),
    }

    for c in range(NCORES):
        m = dict(base_rep)
        srcgm = np.zeros((NB, NCH, 128, KC * 8), np.int16)
        dstgm = np.zeros((NB, NCH, 128, KC * 8), np.int16)
        srcg2m = np.zeros((NB, NCH, 128, KC * 8), np.int16)
        dstg2m = np.zeros((NB, NCH, 128, KC * 8), np.int16)
        ews = np.zeros((NB, NCH, KC, 128), np.float32)
        selTm = np.zeros((NB, NCH, 128, SL), np.float32)
        sel_c = core_of == c
        for w in range(NB):
            es = np.nonzero(sel_c & (win_of == w))[0]
            ns = len(es)
            ssrc = np.zeros(SW, np.int64)
            sdst = np.zeros(SW, np.int64)
            sew = np.zeros(SW, np.float32)
            sloc = np.full(SW, -1, np.int64)
            ssrc[:ns] = s_s[es]
            sdst[:ns] = d_s[es]
            sew[:ns] = w_s[es]
            sloc[:ns] = loc_of[es]
            for ch in range(NCH):
                sl = slice(ch * SL, (ch + 1) * SL)
                srcgm[w, ch] = _wrap_idx(ssrc[sl])
                dstgm[w, ch] = _wrap_idx(sdst[sl])
                srcg2m[w, ch] = _wrap_idx(node_gpad[ssrc[sl]])
                dstg2m[w, ch] = _wrap_idx(node_local[sdst[sl]])
                ews[w, ch] = sew[sl].reshape(KC, 128)
                lc = sloc[sl]
                valid = np.nonzero(lc >= 0)[0]
                tt, pp = valid // 128, valid % 128
                selTm[w, ch, pp, tt * 128 + lc[valid]] = 1.0
        m["srcg"] = np.ascontiguousarray(srcgm.transpose(2, 0, 1, 3))
        m["dstg"] = np.ascontiguousarray(dstgm.transpose(2, 0, 1, 3))
        m["srcg2"] = np.ascontiguousarray(srcg2m.transpose(2, 0, 1, 3))
        m["dstg2"] = np.ascontiguousarray(dstg2m.transpose(2, 0, 1, 3))
        ewt = np.ascontiguousarray(ews.transpose(3, 0, 1, 2))
        m["ew"] = ewt
        m["ewc8"] = np.ascontiguousarray(ewt[..., None] * c1)
        m["selT"] = np.ascontiguousarray(
            selTm.transpose(2, 0, 1, 3)).astype(ml_dtypes.bfloat16)
        in_maps.append(m)

    meta = (node_core, node_local)
    return in_maps, KC, NCH, c2, meta


def kernel(**inputs):
    global LAST_EXEC_NS, LAST_RESULTS
    in_maps, KC, NCH, c2, meta = _prepare(**inputs)
    key = (KC, NCH, c2, _build_program.use_b1, _build_program.use_b2)
    if key not in _CACHE:
        _CACHE[key] = _build_program(KC, NCH)
    nc = _CACHE[key]

    trace = os.environ.get("BASS_GAT_TRACE", "0") == "1"
    res = run_bass_kernel_spmd(nc, in_maps, list(range(NCORES)), trace=trace)
    LAST_EXEC_NS = res.exec_time_ns
    LAST_RESULTS = res
    node_core, node_local = meta
    per_core = [res.results[c]["out_own"] for c in range(NCORES)]
    out = np.empty((N, 4), np.float32)
    for c in range(NCORES):
        mask = node_core == c
        out[mask] = per_core[c][node_local[mask]]
    return out


# revision 11
# speedup vs baseline: 1.4395x; 1.2662x over previous
"""Trainium2 Bass kernel for a 2-layer GAT (nn_GAT_781684048444).

Strategy (8 NeuronCores, SPMD) — v2:
  - Nodes assigned to 80 windows (8 cores x 10 windows x 128 slots) by greedy
    in-degree balancing; edges grouped by dst window, padded to K 128-edge
    tiles; one static program serves all cores. Output inverse-permuted.
  - Stage 0 (replicated): ONE bf16 matmul per 128-node block computes the
    packed row [512 bf16 msg (head-minor) | 8 fp32 s_src | 8 fp32 s_dst]
    (attention dots folded into the weight matrix; the fp32 PSUM result is
    staged to SBUF once, msg cols reach DRAM through a gpsimd cast-DMA
    (fp32->bf16 in the DMA), scalar cols through a raw bit-copy DMA on SP).
    No separate fp32 x load, no second matmul, no big DVE copy traffic.
  - Layer-1 edge phase: gather 1280B rows by src + 256B scalar blocks by dst;
    alpha adds run on gpsimd, leaky-relu on DVE, exp on ACT; the bf16
    msg *= alpha broadcast runs in DVE 2x mode; scatter-add via resident
    one-hot selT bf16 matmuls into PSUM. h1 = elu(U/D) with the elu expressed
    as relu/exp on ACT (elu(x) = relu(x) + exp(-relu(-x)) - 1), only the
    final combine on DVE. h1 kept bf16.
  - Layer-2 projection per window (bf16 transpose + matmul) feeds a compact
    [NPAD, 8] fp32 table; the AllGather is SPLIT in two (windows 0-4 issued
    mid-layer-1, 5-9 at the end) so most of its fixed cost overlaps layer 1.
    dst-side gathers + alpha partials are computed in the shadow of the
    second collective; src gathers + the final gather-attend-scatter follow.
"""

import os
import sys

import ml_dtypes
import numpy as np

sys.path.insert(0, "/opt/trn_rl_repo")

from concourse import bacc, bass, mybir, tile  # noqa: E402
from concourse.bass import AP  # noqa: E402
from concourse.bass_utils import run_bass_kernel_spmd  # noqa: E402

N, E = 10000, 160000
IN, HID, OUT, H = 128, 64, 4, 8
C1 = H * HID               # 512 layer-1 out width
TMW = 320                  # fp32 row: 256 (512 bf16 msg) | 8 ssrc | 8 sdst | 48 pad
T2W = 64                   # layer-2 expanded row width (fp32)
NCORES = 8
NPC = N // NCORES
WIN = 128
NB = 10
NPAD = NB * WIN            # 1280
NWIN = NCORES * NB         # 80
NBLK = (N + 127) // 128    # 79
MAXI = 1024
WSPLIT = 5                 # windows in first collective slice

FP = mybir.dt.float32
BF = mybir.dt.bfloat16
I16 = mybir.dt.int16

_CACHE = {}

LAST_EXEC_NS = None
LAST_RESULTS = None


def _wrap_idx(vals):
    """int16 gather index layout: idx i -> [i%16, i//16], tiled to 128 partitions."""
    n = vals.shape[0]
    w = np.zeros((16, n // 16), np.int16)
    w[np.arange(n) % 16, np.arange(n) // 16] = vals.astype(np.int16)
    return np.tile(w, (8, 1))


def _build_program(KC, NCH):
    K = KC * NCH
    SL = KC * 128

    nc = bacc.Bacc("TRN2", target_bir_lowering=False, debug=False, num_devices=NCORES)

    # ---- DRAM parameters ----
    xTb_d = nc.dram_tensor("xTb", [IN, N], BF, kind="ExternalInput")
    W1sd_d = nc.dram_tensor("W1sd", [IN, 528], BF, kind="ExternalInput")
    W2_d = nc.dram_tensor("W2r", [128, 4, 8], BF, kind="ExternalInput")
    b2_d = nc.dram_tensor("b2rep", [128, 8], FP, kind="ExternalInput")
    b1_d = nc.dram_tensor("b1rep", [128, C1], FP, kind="ExternalInput")
    ident_d = nc.dram_tensor("identb", [128, 128], BF, kind="ExternalInput")
    ewc8_d = nc.dram_tensor("ewc8", [128, NB, NCH, KC, 8], FP, kind="ExternalInput")
    ewc2_d = nc.dram_tensor("ewc2", [128, NB, NCH, KC], FP, kind="ExternalInput")
    srcg_d = nc.dram_tensor("srcg", [128, NB, NCH, KC * 8], I16, kind="ExternalInput")
    dstg_d = nc.dram_tensor("dstg", [128, NB, NCH, KC * 8], I16, kind="ExternalInput")
    srcg2_d = nc.dram_tensor("srcg2", [128, NB, NCH, KC * 8], I16, kind="ExternalInput")
    dstg2_d = nc.dram_tensor("dstg2", [128, NB, NCH, KC * 8], I16, kind="ExternalInput")
    selT_d = nc.dram_tensor("selT", [128, NB, NCH, SL], BF, kind="ExternalInput")

    out_d = nc.dram_tensor("out_own", [NPAD, 4], FP, kind="ExternalOutput")

    # ---- internal DRAM ----
    tableM = nc.dram_tensor("tableM", [NBLK * 128, TMW], FP)
    table2x = nc.dram_tensor("table2x", [NPAD, T2W], FP)
    table2c = nc.dram_tensor("table2c", [NPAD, 4], FP)
    t2cf = nc.dram_tensor("t2cf", [NCORES * NPAD, 4], FP, addr_space="Shared")
    table2f = nc.dram_tensor("table2f", [NCORES * NPAD, T2W], FP)

    c2_host = _build_program.c2_host
    XCH = 10

    with tile.TileContext(nc) as tc:
        with (
            tc.tile_pool(name="const", bufs=1) as constp,
            tc.tile_pool(name="idx", bufs=1) as idxp,
            tc.tile_pool(name="h1p", bufs=1) as h1p,
            tc.tile_pool(name="selp", bufs=1) as selp,
        ):
            W1sd = constp.tile([IN, 528], BF)
            nc.scalar.dma_start(W1sd[:], W1sd_d[:])
            W2sb = constp.tile([128, 4, 8], BF)
            nc.scalar.dma_start(W2sb[:], W2_d[:])
            b2rep = constp.tile([128, 8], FP)
            nc.scalar.dma_start(b2rep[:], b2_d[:])
            if _build_program.use_b1:
                b1rep = constp.tile([128, C1], FP)
                nc.scalar.dma_start(b1rep[:], b1_d[:])
            identb = constp.tile([128, 128], BF)
            nc.scalar.dma_start(identb[:], ident_d[:])
            ewc8sb = constp.tile([128, NB, NCH, KC, 8], FP)
            ewsb = constp.tile([128, NB, NCH, KC], FP)
            srcg = idxp.tile([128, NB, NCH, KC * 8], I16)
            dstg = idxp.tile([128, NB, NCH, KC * 8], I16)
            srcg2 = idxp.tile([128, NB, NCH, KC * 8], I16)
            dstg2 = idxp.tile([128, NB, NCH, KC * 8], I16)
            selsb = selp.tile([128, NB, NCH, SL], BF)
            h1own = h1p.tile([128, NB, C1], BF)

            # ========== stage 0: packed table via cast-DMA ==========
            with (
                tc.tile_pool(name="s0x", bufs=8) as s0xp,
                tc.tile_pool(name="s0stg", bufs=3) as s0stgp,
                tc.tile_pool(name="s0ps", bufs=4, space="PSUM") as s0ps,
            ):
                xbch = []
                for xc in range((NBLK + XCH - 1) // XCH):
                    c0 = xc * XCH * 128
                    cz = min(N, (xc + 1) * XCH * 128)
                    xb = s0xp.tile([IN, XCH * 128], BF, tag="xb")
                    nc.sync.dma_start(xb[:, : cz - c0], xTb_d[:, c0:cz])
                    xbch.append(xb)
                # L1 metadata queues behind x on SP
                nc.sync.dma_start(srcg[:], srcg_d[:])
                nc.sync.dma_start(dstg[:], dstg_d[:])
                nc.sync.dma_start(ewc8sb[:], ewc8_d[:])

                stg = None
                for b in range(NBLK):
                    rows = min(128, N - b * 128)
                    off = (b % XCH) * 128
                    psAB = s0ps.tile([128, 1024], FP, tag="psAB")
                    xsl = xbch[b // XCH][:, off : off + rows]
                    nc.tensor.matmul(psAB[:rows, 0:512], xsl, W1sd[:, 0:512],
                                     start=True, stop=True)
                    nc.tensor.matmul(psAB[:rows, 512:528], xsl, W1sd[:, 512:528],
                                     start=True, stop=True)
                    if b % 4 == 0:
                        stg = s0stgp.tile([128, 4, 528], FP, tag="stg")
                    bi = b % 4
                    if b == NBLK - 1 and rows < 128:
                        nc.vector.memset(stg[:, bi, :], 0.0)
                    if b % 2 == 0:
                        nc.vector.tensor_copy(stg[:rows, bi, :], psAB[:rows, 0:528])
                    else:
                        nc.scalar.copy(stg[:rows, bi, :], psAB[:rows, 0:528])
                    if bi == 3 or b == NBLK - 1:
                        gsz = bi + 1
                        b0 = b - bi
                        outM = AP(tableM[:].tensor, b0 * 128 * TMW,
                                  [(TMW, 128), (128 * TMW, gsz), (1, 264)]).bitcast(BF)
                        nc.gpsimd.dma_start(outM, stg[:, 0:gsz, 0:528])

            # ================= layer 1 edge phase =================
            # Software-pipelined: window w's finalize is split around window
            # w+1's chunk work so the ACT elu chain and PSUM reads hide under
            # the next window's gathers and msg-multiplies.
            with (
                tc.tile_pool(name="g1", bufs=3) as g1p,
                tc.tile_pool(name="gd1", bufs=3) as gd1p,
                tc.tile_pool(name="al1", bufs=3) as al1p,
                tc.tile_pool(name="wend", bufs=2) as wendp,
                tc.tile_pool(name="l2h", bufs=3) as l2hp,
                tc.tile_pool(name="gd2", bufs=1) as gd2p,
                tc.tile_pool(name="ps1", bufs=2, space="PSUM") as ps1p,
                tc.tile_pool(name="l2ps", bufs=2, space="PSUM") as l2ps,
                tc.tile_pool(name="l2tp", bufs=2, space="PSUM") as l2tp,
            ):
                # remaining L2 metadata on SP early in L1
                nc.sync.dma_start(srcg2[:], srcg2_d[:])
                nc.sync.dma_start(dstg2[:], dstg2_d[:])
                nc.sync.dma_start(ewsb[:], ewc2_d[:])

                gd2all = gd2p.tile([128, NB, NCH, KC, T2W], FP)

                def chunks(w, psU, psD):
                    nc.sync.dma_start(selsb[:, w], selT_d[:, w])
                    for ch in range(NCH):
                        g = g1p.tile([128, KC, TMW], FP)
                        nc.gpsimd.dma_gather(
                            g[:], tableM[:], srcg[:, w, ch, :], SL, SL, TMW
                        )
                        gd = gd1p.tile([128, KC, 64], FP)
                        nc.gpsimd.dma_gather(
                            gd[:], tableM[:, 256:320], dstg[:, w, ch, :], SL, SL, 64,
                            elem_step=TMW,
                        )
                        gb = g[:, :, 256:264].bitcast(BF)    # [128, KC, 16]
                        gdb = gd[:, :, 0:8].bitcast(BF)      # [128, KC, 16]
                        a = al1p.tile([128, KC, 8], FP)
                        nc.gpsimd.tensor_tensor(
                            out=a[:], in0=gb[:, :, 0:8], in1=gdb[:, :, 8:16],
                            op=mybir.AluOpType.add,
                        )
                        nc.gpsimd.tensor_tensor(
                            out=a[:], in0=a[:], in1=ewc8sb[:, w, ch],
                            op=mybir.AluOpType.add,
                        )
                        nc.vector.scalar_tensor_tensor(
                            out=a[:], in0=a[:], scalar=0.2, in1=a[:],
                            op0=mybir.AluOpType.mult, op1=mybir.AluOpType.max)
                        ahb = al1p.tile([128, KC, 1, 8], BF)
                        nc.scalar.activation(ahb[:, :, 0, :], a[:],
                                             mybir.ActivationFunctionType.Exp)
                        mv = g[:, :, 0:256].bitcast(BF)
                        msg4 = mv.rearrange("p t (c h) -> p t c h", h=8)
                        ah4 = ahb[:].to_broadcast([128, KC, 64, 8])
                        nc.vector.tensor_tensor(out=msg4, in0=msg4, in1=ah4,
                                                op=mybir.AluOpType.mult)
                        for t in range(KC):
                            ti = ch * KC + t
                            st = ti == 0
                            sp = ti == K - 1
                            sel = selsb[:, w, ch, t * 128 : (t + 1) * 128]
                            nc.tensor.matmul(psU[:], sel, g[:, t, 0:256].bitcast(BF),
                                             start=st, stop=sp)
                            nc.tensor.matmul(psD[:], sel, ahb[:, t, 0, :],
                                             start=st, stop=sp)

                def fin1(w, psU, psD):
                    # h1 = U/D in bf16 + start of the ACT elu chain
                    dpe = wendp.tile([128, 8], FP, tag="dpe")
                    nc.vector.tensor_scalar_add(dpe[:], psD[:], 1e-16)
                    dr = wendp.tile([128, 1, 8], FP, tag="dr")
                    nc.vector.reciprocal(dr[:, 0, :], dpe[:])
                    h1v = h1own[:, w, :]
                    h1v3 = h1v.rearrange("p (c h) -> p c h", h=8)
                    psU3 = psU[:].rearrange("p (c h) -> p c h", h=8)
                    nc.vector.tensor_tensor(out=h1v3, in0=psU3,
                                            in1=dr[:].to_broadcast([128, 64, 8]),
                                            op=mybir.AluOpType.mult)
                    if _build_program.use_b1:
                        nc.vector.tensor_tensor(out=h1v, in0=h1v, in1=b1rep[:],
                                                op=mybir.AluOpType.add)
                    negp = wendp.tile([128, C1], BF, tag="negp")
                    nc.scalar.activation(negp[:], h1v,
                                         mybir.ActivationFunctionType.Relu,
                                         scale=-1.0)
                    emin = wendp.tile([128, C1], BF, tag="emin")
                    nc.scalar.activation(emin[:], negp[:],
                                         mybir.ActivationFunctionType.Exp,
                                         scale=-1.0)
                    posp = wendp.tile([128, C1], BF, tag="posp")
                    nc.scalar.activation(posp[:], h1v,
                                         mybir.ActivationFunctionType.Relu)
                    return posp, emin

                def fin2(w, posp, emin):
                    # elu combine + layer-2 projection + dst-side L2 gathers
                    h1v = h1own[:, w, :]
                    nc.vector.scalar_tensor_tensor(
                        out=h1v, in0=posp[:], scalar=-1.0, in1=emin[:],
                        op0=mybir.AluOpType.add, op1=mybir.AluOpType.add,
                    )
                    ps2 = l2ps.tile([128, 8], FP)
                    for kc in range(4):
                        tps = l2tp.tile([128, 128], BF)
                        nc.tensor.transpose(
                            tps[:], h1own[:, w, kc * 128 : (kc + 1) * 128], identb[:])
                        tsb = l2hp.tile([128, 128], BF, tag="tsb")
                        if kc % 2 == 0:
                            nc.vector.tensor_copy(tsb[:], tps[:])
                        else:
                            nc.scalar.copy(tsb[:], tps[:])
                        nc.tensor.matmul(ps2[:], tsb[:], W2sb[:, kc, :],
                                         start=(kc == 0), stop=(kc == 3))
                    # bf16-packed row: [4 h2 | s2src | s2dst | pad] = 16B
                    st2b = l2hp.tile([128, 8], BF, tag="st2b")
                    nc.scalar.copy(st2b[:], ps2[:])
                    st2f = st2b[:].bitcast(FP)              # [128, 4]
                    nc.sync.dma_start(table2c[w * 128 : (w + 1) * 128, :], st2f)
                    out2x = AP(table2x[:].tensor, w * 128 * T2W,
                               [(T2W, 128), (1, 4)])
                    nc.sync.dma_start(out2x, st2f)
                    for ch in range(NCH):
                        nc.gpsimd.dma_gather(
                            gd2all[:, w, ch], table2x[:], dstg2[:, w, ch, :],
                            SL, SL, T2W,
                        )

                prev = None
                for w in range(NB):
                    psU = ps1p.tile([128, 512], FP, tag="psU")
                    psD = ps1p.tile([128, 8], FP, tag="psD")
                    if prev is not None:
                        pw, pU, pD = prev
                        pp, pe = fin1(pw, pU, pD)
                    chunks(w, psU, psD)
                    if prev is not None:
                        fin2(pw, pp, pe)
                    prev = (w, psU, psD)
                pw, pU, pD = prev
                pp, pe = fin1(pw, pU, pD)
                fin2(pw, pp, pe)

                # single AllGather of the bf16-packed compact table (16B rows)
                nc.gpsimd.collective_compute(
                    "AllGather", mybir.AluOpType.bypass,
                    replica_groups=[list(range(NCORES))],
                    ins=[table2c[:]],
                    outs=[t2cf[:]],
                )
                # expansion: drop rows into cols 0:4 of table2f
                nc.sync.dma_start(table2f[:, 0:4], t2cf[:])

            # ================= layer 2 edge phase =================
            with (
                tc.tile_pool(name="g2", bufs=3) as g2p,
                tc.tile_pool(name="al2", bufs=3) as al2p,
                tc.tile_pool(name="wend2", bufs=2) as wend2p,
                tc.tile_pool(name="ps2p", bufs=2, space="PSUM") as ps2pp,
            ):
                for w in range(NB):
                    psO = ps2pp.tile([128, 8], FP)
                    for ch in range(NCH):
                        gs = g2p.tile([128, KC, T2W], FP)
                        nc.gpsimd.dma_gather(
                            gs[:], table2f[:], srcg2[:, w, ch, :], SL, SL, T2W
                        )
                        gsb = gs[:, :, 0:4].bitcast(BF)      # [128, KC, 8]
                        gdb2 = gd2all[:, w, ch, :, 2:4].bitcast(BF)  # [128, KC, 4]
                        a2 = al2p.tile([128, KC, 1], FP, tag="a2")
                        nc.vector.tensor_tensor(out=a2[:], in0=gsb[:, :, 4:5],
                                                in1=gdb2[:, :, 1:2],
                                                op=mybir.AluOpType.add)
                        ewc2_b = ewsb[:, w, ch, :].to_broadcast([128, KC, 1])
                        nc.vector.tensor_tensor(out=a2[:], in0=a2[:], in1=ewc2_b,
                                                op=mybir.AluOpType.add)
                        nc.vector.scalar_tensor_tensor(
                            out=a2[:], in0=a2[:], scalar=0.2, in1=a2[:],
                            op0=mybir.AluOpType.mult, op1=mybir.AluOpType.max)
                        g5b = al2p.tile([128, KC, 8], BF, tag="g5b")
                        nc.vector.tensor_copy(g5b[:, :, 0:4], gsb[:, :, 0:4])
                        nc.scalar.activation(g5b[:, :, 4:5], a2[:],
                                             mybir.ActivationFunctionType.Exp)
                        ah = g5b[:, :, 4:5].to_broadcast([128, KC, 4])
                        nc.vector.tensor_tensor(out=g5b[:, :, 0:4],
                                                in0=g5b[:, :, 0:4],
                                                in1=ah, op=mybir.AluOpType.mult)
                        for t in range(KC):
                            ti = ch * KC + t
                            sel = selsb[:, w, ch, t * 128 : (t + 1) * 128]
                            nc.tensor.matmul(psO[:, 0:5], sel, g5b[:, t, 0:5],
                                             start=(ti == 0), stop=(ti == K - 1))
                    dpe = wend2p.tile([128, 1], FP, tag="dpe2")
                    nc.vector.tensor_scalar_add(dpe[:], psO[:, 4:5], 1e-16)
                    dr = wend2p.tile([128, 1], FP, tag="dr2")
                    nc.vector.reciprocal(dr[:], dpe[:])
                    ob = wend2p.tile([128, 8], FP, tag="ob")
                    nc.vector.tensor_tensor(out=ob[:, 0:4], in0=psO[:, 0:4],
                                            in1=dr[:].to_broadcast([128, 4]),
                                            op=mybir.AluOpType.mult)
                    if _build_program.use_b2:
                        nc.vector.tensor_tensor(out=ob[:, 0:4], in0=ob[:, 0:4],
                                                in1=b2rep[:, 0:4],
                                                op=mybir.AluOpType.add)
                    nc.sync.dma_start(out_d[w * 128 : (w + 1) * 128, :], ob[:, 0:4])

    nc.compile()
    return nc


def _balance_windows(dst):
    """Greedy in-degree balancing of nodes into NWIN windows of WIN slots."""
    import heapq

    indeg = np.bincount(dst, minlength=N)
    order = np.argsort(-indeg, kind="stable")
    heap = [(0, w) for w in range(NWIN)]
    heapq.heapify(heap)
    fill = np.zeros(NWIN, np.int64)
    node_win = np.zeros(N, np.int64)
    node_slot = np.zeros(N, np.int64)
    for n in order:
        cnt, w = heapq.heappop(heap)
        node_win[n] = w
        node_slot[n] = fill[w]
        fill[w] += 1
        if fill[w] < WIN:
            heapq.heappush(heap, (cnt + int(indeg[n]), w))
    return node_win, node_slot


def _prepare(x, edge_index, edge_weight, W1, att_src1, att_dst1, att_edge1, We1, b1,
             W2, att_src2, att_dst2, att_edge2, We2, b2):
    x = np.asarray(x, np.float32)
    ei = np.asarray(edge_index)
    ew = np.asarray(edge_weight, np.float32)
    W1 = np.asarray(W1, np.float32)
    att_src1 = np.asarray(att_src1, np.float32)
    att_dst1 = np.asarray(att_dst1, np.float32)
    att_edge1 = np.asarray(att_edge1, np.float32)
    We1 = np.asarray(We1, np.float32)
    b1 = np.asarray(b1, np.float32)
    W2 = np.asarray(W2, np.float32)
    att_src2 = np.asarray(att_src2, np.float32)
    att_dst2 = np.asarray(att_dst2, np.float32)
    att_edge2 = np.asarray(att_edge2, np.float32)
    We2 = np.asarray(We2, np.float32)
    b2 = np.asarray(b2, np.float32)

    # ---------- weight folding ----------
    W1r = W1.reshape(IN, H, HID)
    Wsrc = np.einsum("khc,hc->kh", W1r, att_src1)
    Wdst = np.einsum("khc,hc->kh", W1r, att_dst1)
    c1 = (We1.reshape(H, HID) * att_edge1).sum(1).astype(np.float32)  # [H]

    # head-minor column order: new col c*8+h = old h*64+c
    cols = np.tile(np.arange(H), HID) * HID + np.repeat(np.arange(HID), H)
    W1p = np.ascontiguousarray(W1[:, cols])
    b1p = b1[cols]
    W2p = W2[cols, :]
    W1sd = np.concatenate([W1p, Wsrc, Wdst], axis=1)  # [IN, 528]

    Waug2 = np.zeros((C1, 8), np.float32)
    Waug2[:, 0:4] = W2p
    Waug2[:, 4] = W2p @ att_src2[0]
    Waug2[:, 5] = W2p @ att_dst2[0]
    W2resh = np.ascontiguousarray(Waug2.reshape(4, 128, 8).transpose(1, 0, 2))
    c2 = float((We2[0] * att_edge2[0]).sum())
    _build_program.c2_host = c2
    _build_program.use_b1 = bool(np.any(b1))
    _build_program.use_b2 = bool(np.any(b2))

    # ---------- edge partitioning ----------
    src = np.asarray(ei[0], np.int64)
    dst = np.asarray(ei[1], np.int64)

    node_win, node_slot = _balance_windows(dst)
    node_core = node_win // NB
    node_w = node_win % NB
    node_local = node_w * WIN + node_slot
    node_gpad = node_core * NPAD + node_local

    ekey = node_win[dst]
    order = np.argsort(ekey, kind="stable")
    s_s, d_s, w_s = src[order], dst[order], ew[order]
    core_of = node_core[d_s]
    win_of = node_w[d_s]
    loc_of = node_slot[d_s]

    cnt = np.bincount(node_win[d_s], minlength=NWIN)
    K = int(np.ceil(cnt.max() / 128.0))
    NCHo = os.environ.get("BASS_GAT_NCH")
    if NCHo is not None:
        NCH = int(NCHo)
        KC = (K + NCH - 1) // NCH
    else:
        NCH = 2
        while ((K + NCH - 1) // NCH) * 128 > MAXI:
            NCH += 1
        KC = (K + NCH - 1) // NCH
    K = KC * NCH
    SL = KC * 128
    SW = K * 128

    in_maps = []
    base_rep = {
        "xTb": np.ascontiguousarray(x.T).astype(ml_dtypes.bfloat16),
        "W1sd": W1sd.astype(ml_dtypes.bfloat16),
        "W2r": W2resh.astype(ml_dtypes.bfloat16),
        "b2rep": np.tile(np.concatenate([b2, np.zeros(4, np.float32)])[None, :],
                         (128, 1)),
        "b1rep": np.tile(b1p[None, :], (128, 1)),
        "identb": np.eye(128, dtype=np.float32).astype(ml_dtypes.bfloat16),
    }

    for c in range(NCORES):
        m = dict(base_rep)
        srcgm = np.zeros((NB, NCH, 128, KC * 8), np.int16)
        dstgm = np.zeros((NB, NCH, 128, KC * 8), np.int16)
        srcg2m = np.zeros((NB, NCH, 128, KC * 8), np.int16)
        dstg2m = np.zeros((NB, NCH, 128, KC * 8), np.int16)
        ews = np.zeros((NB, NCH, KC, 128), np.float32)
        selTm = np.zeros((NB, NCH, 128, SL), np.float32)
        sel_c = core_of == c
        for w in range(NB):
            es = np.nonzero(sel_c & (win_of == w))[0]
            ns = len(es)
            ssrc = np.zeros(SW, np.int64)
            sdst = np.zeros(SW, np.int64)
            sew = np.zeros(SW, np.float32)
            sloc = np.full(SW, -1, np.int64)
            ssrc[:ns] = s_s[es]
            sdst[:ns] = d_s[es]
            sew[:ns] = w_s[es]
            sloc[:ns] = loc_of[es]
            for ch in range(NCH):
                sl = slice(ch * SL, (ch + 1) * SL)
                srcgm[w, ch] = _wrap_idx(ssrc[sl])
                dstgm[w, ch] = _wrap_idx(sdst[sl])
                srcg2m[w, ch] = _wrap_idx(node_gpad[ssrc[sl]])
                dstg2m[w, ch] = _wrap_idx(node_local[sdst[sl]])
                ews[w, ch] = sew[sl].reshape(KC, 128)
                lc = sloc[sl]
                valid = np.nonzero(lc >= 0)[0]
                tt, pp = valid // 128, valid % 128
                selTm[w, ch, pp, tt * 128 + lc[valid]] = 1.0
        m["srcg"] = np.ascontiguousarray(srcgm.transpose(2, 0, 1, 3))
        m["dstg"] = np.ascontiguousarray(dstgm.transpose(2, 0, 1, 3))
        m["srcg2"] = np.ascontiguousarray(srcg2m.transpose(2, 0, 1, 3))
        m["dstg2"] = np.ascontiguousarray(dstg2m.transpose(2, 0, 1, 3))
        ewt = np.ascontiguousarray(ews.transpose(3, 0, 1, 2))
        m["ewc2"] = ewt * c2
        m["ewc8"] = np.ascontiguousarray(ewt[..., None] * c1)
        m["selT"] = np.ascontiguousarray(
            selTm.transpose(2, 0, 1, 3)).astype(ml_dtypes.bfloat16)
        in_maps.append(m)

    meta = (node_core, node_local)
    return in_maps, KC, NCH, c2, meta


def kernel(**inputs):
    global LAST_EXEC_NS, LAST_RESULTS
    in_maps, KC, NCH, c2, meta = _prepare(**inputs)
    key = (KC, NCH, c2, _build_program.use_b1, _build_program.use_b2)
    if key not in _CACHE:
        _CACHE[key] = _build_program(KC, NCH)
    nc = _CACHE[key]

    trace = os.environ.get("BASS_GAT_TRACE", "0") == "1"
    res = run_bass_kernel_spmd(nc, in_maps, list(range(NCORES)), trace=trace)
    LAST_EXEC_NS = res.exec_time_ns
    LAST_RESULTS = res
    node_core, node_local = meta
    per_core = [res.results[c]["out_own"] for c in range(NCORES)]
    out = np.empty((N, 4), np.float32)
    for c in range(NCORES):
        mask = node_core == c
        out[mask] = per_core[c][node_local[mask]]
    return out


# revision 12
# speedup vs baseline: 1.4986x; 1.0410x over previous
"""Trainium2 Bass kernel for a 2-layer GAT (nn_GAT_781684048444).

Strategy (8 NeuronCores, SPMD) — v2:
  - Nodes assigned to 80 windows (8 cores x 10 windows x 128 slots) by greedy
    in-degree balancing; edges grouped by dst window, padded to K 128-edge
    tiles; one static program serves all cores. Output inverse-permuted.
  - Stage 0 (replicated): ONE bf16 matmul per 128-node block computes the
    packed row [512 bf16 msg (head-minor) | 8 fp32 s_src | 8 fp32 s_dst]
    (attention dots folded into the weight matrix; the fp32 PSUM result is
    staged to SBUF once, msg cols reach DRAM through a gpsimd cast-DMA
    (fp32->bf16 in the DMA), scalar cols through a raw bit-copy DMA on SP).
    No separate fp32 x load, no second matmul, no big DVE copy traffic.
  - Layer-1 edge phase: gather 1280B rows by src + 256B scalar blocks by dst;
    alpha adds run on gpsimd, leaky-relu on DVE, exp on ACT; the bf16
    msg *= alpha broadcast runs in DVE 2x mode; scatter-add via resident
    one-hot selT bf16 matmuls into PSUM. h1 = elu(U/D) with the elu expressed
    as relu/exp on ACT (elu(x) = relu(x) + exp(-relu(-x)) - 1), only the
    final combine on DVE. h1 kept bf16.
  - Layer-2 projection per window (bf16 transpose + matmul) feeds a compact
    [NPAD, 8] fp32 table; the AllGather is SPLIT in two (windows 0-4 issued
    mid-layer-1, 5-9 at the end) so most of its fixed cost overlaps layer 1.
    dst-side gathers + alpha partials are computed in the shadow of the
    second collective; src gathers + the final gather-attend-scatter follow.
"""

import os
import sys

import ml_dtypes
import numpy as np

sys.path.insert(0, "/opt/trn_rl_repo")

from concourse import bacc, bass, mybir, tile  # noqa: E402
from concourse.bass import AP  # noqa: E402
from concourse.bass_utils import run_bass_kernel_spmd  # noqa: E402

N, E = 10000, 160000
IN, HID, OUT, H = 128, 64, 4, 8
C1 = H * HID               # 512 layer-1 out width
TMW = 320                  # fp32 row: 256 (512 bf16 msg) | 8 ssrc | 8 sdst | 48 pad
T2W = 64                   # layer-2 expanded row width (fp32)
NCORES = 8
NPC = N // NCORES
WIN = 128
NB = 10
NPAD = NB * WIN            # 1280
NWIN = NCORES * NB         # 80
NBLK = (N + 127) // 128    # 79
MAXI = 1024
WSPLIT = 5                 # windows in first collective slice

FP = mybir.dt.float32
BF = mybir.dt.bfloat16
I16 = mybir.dt.int16

_CACHE = {}

LAST_EXEC_NS = None
LAST_RESULTS = None


def _wrap_idx(vals):
    """int16 gather index layout: idx i -> [i%16, i//16], tiled to 128 partitions."""
    n = vals.shape[0]
    w = np.zeros((16, n // 16), np.int16)
    w[np.arange(n) % 16, np.arange(n) // 16] = vals.astype(np.int16)
    return np.tile(w, (8, 1))


def _build_program(KC, NCH):
    K = KC * NCH
    SL = KC * 128

    nc = bacc.Bacc("TRN2", target_bir_lowering=False, debug=False, num_devices=NCORES)

    # ---- DRAM parameters ----
    xTb_d = nc.dram_tensor("xTb", [IN, N], BF, kind="ExternalInput")
    W1sd_d = nc.dram_tensor("W1sd", [IN, 528], BF, kind="ExternalInput")
    W2_d = nc.dram_tensor("W2r", [128, 4, 8], BF, kind="ExternalInput")
    b2_d = nc.dram_tensor("b2rep", [128, 8], FP, kind="ExternalInput")
    b1_d = nc.dram_tensor("b1rep", [128, C1], FP, kind="ExternalInput")
    ident_d = nc.dram_tensor("identb", [128, 128], BF, kind="ExternalInput")
    ewc8_d = nc.dram_tensor("ewc8", [128, NB, NCH, KC, 8], FP, kind="ExternalInput")
    ewc2_d = nc.dram_tensor("ewc2", [128, NB, NCH, KC], FP, kind="ExternalInput")
    srcg_d = nc.dram_tensor("srcg", [128, NB, NCH, KC * 8], I16, kind="ExternalInput")
    dstg_d = nc.dram_tensor("dstg", [128, NB, NCH, KC * 8], I16, kind="ExternalInput")
    srcg2_d = nc.dram_tensor("srcg2", [128, NB, NCH, KC * 8], I16, kind="ExternalInput")
    dstg2_d = nc.dram_tensor("dstg2", [128, NB, NCH, KC * 8], I16, kind="ExternalInput")
    selT_d = nc.dram_tensor("selT", [128, NB, NCH, SL], BF, kind="ExternalInput")

    out_d = nc.dram_tensor("out_own", [NPAD, 4], FP, kind="ExternalOutput")

    # ---- internal DRAM ----
    tableM = nc.dram_tensor("tableM", [NBLK * 128, TMW], FP)
    table2x = nc.dram_tensor("table2x", [NPAD, T2W], FP)
    table2c = nc.dram_tensor("table2c", [NPAD, 4], FP)
    t2cf = nc.dram_tensor("t2cf", [NCORES * NPAD, 4], FP, addr_space="Shared")
    table2f = nc.dram_tensor("table2f", [NCORES * NPAD, T2W], FP)

    c2_host = _build_program.c2_host
    XCH = 10

    with tile.TileContext(nc) as tc:
        with (
            tc.tile_pool(name="const", bufs=1) as constp,
            tc.tile_pool(name="idx", bufs=1) as idxp,
            tc.tile_pool(name="h1p", bufs=1) as h1p,
            tc.tile_pool(name="selp", bufs=1) as selp,
        ):
            W1sd = constp.tile([IN, 528], BF)
            nc.scalar.dma_start(W1sd[:], W1sd_d[:])
            W2sb = constp.tile([128, 4, 8], BF)
            nc.scalar.dma_start(W2sb[:], W2_d[:])
            b2rep = constp.tile([128, 8], FP)
            nc.scalar.dma_start(b2rep[:], b2_d[:])
            if _build_program.use_b1:
                b1rep = constp.tile([128, C1], FP)
                nc.scalar.dma_start(b1rep[:], b1_d[:])
            identb = constp.tile([128, 128], BF)
            nc.scalar.dma_start(identb[:], ident_d[:])
            ewc8sb = constp.tile([128, NB, NCH, KC, 8], FP)
            ewsb = constp.tile([128, NB, NCH, KC], FP)
            srcg = idxp.tile([128, NB, NCH, KC * 8], I16)
            dstg = idxp.tile([128, NB, NCH, KC * 8], I16)
            srcg2 = idxp.tile([128, NB, NCH, KC * 8], I16)
            dstg2 = idxp.tile([128, NB, NCH, KC * 8], I16)
            selsb = selp.tile([128, NB, NCH, SL], BF)
            h1own = h1p.tile([128, NB, C1], BF)

            # ========== stage 0: packed table via cast-DMA ==========
            with (
                tc.tile_pool(name="s0x", bufs=8) as s0xp,
                tc.tile_pool(name="s0stg", bufs=2) as s0stgp,
                tc.tile_pool(name="s0stgb", bufs=2) as s0stgbp,
                tc.tile_pool(name="s0ps", bufs=4, space="PSUM") as s0ps,
            ):
                xbch = []
                for xc in range((NBLK + XCH - 1) // XCH):
                    c0 = xc * XCH * 128
                    cz = min(N, (xc + 1) * XCH * 128)
                    xb = s0xp.tile([IN, XCH * 128], BF, tag="xb")
                    nc.sync.dma_start(xb[:, : cz - c0], xTb_d[:, c0:cz])
                    xbch.append(xb)
                # L1 metadata queues behind x on SP
                nc.sync.dma_start(srcg[:], srcg_d[:])
                nc.sync.dma_start(dstg[:], dstg_d[:])
                nc.sync.dma_start(ewc8sb[:], ewc8_d[:])

                stg = None
                for b in range(NBLK):
                    rows = min(128, N - b * 128)
                    off = (b % XCH) * 128
                    grp = b // 4
                    sp_route = grp % 5 in (2, 4)  # 8/20 groups: bf16 staging + SP write
                    psAB = s0ps.tile([128, 1024], FP, tag="psAB")
                    xsl = xbch[b // XCH][:, off : off + rows]
                    nc.tensor.matmul(psAB[:rows, 0:512], xsl, W1sd[:, 0:512],
                                     start=True, stop=True)
                    nc.tensor.matmul(psAB[:rows, 512:528], xsl, W1sd[:, 512:528],
                                     start=True, stop=True)
                    if b % 4 == 0:
                        if sp_route:
                            stg = s0stgbp.tile([128, 4, 528], BF, tag="stgb")
                        else:
                            stg = s0stgp.tile([128, 4, 528], FP, tag="stg")
                    bi = b % 4
                    if b == NBLK - 1 and rows < 128:
                        nc.vector.memset(stg[:, bi, :], 0.0)
                    if b % 2 == 0:
                        nc.vector.tensor_copy(stg[:rows, bi, :], psAB[:rows, 0:528])
                    else:
                        nc.scalar.copy(stg[:rows, bi, :], psAB[:rows, 0:528])
                    if bi == 3 or b == NBLK - 1:
                        gsz = bi + 1
                        b0 = b - bi
                        outM = AP(tableM[:].tensor, b0 * 128 * TMW,
                                  [(TMW, 128), (128 * TMW, gsz), (1, 264)]).bitcast(BF)
                        if sp_route:
                            nc.sync.dma_start(outM, stg[:, 0:gsz, :])
                        else:
                            nc.gpsimd.dma_start(outM, stg[:, 0:gsz, 0:528])

            # ================= layer 1 edge phase =================
            # Software-pipelined: window w's finalize is split around window
            # w+1's chunk work so the ACT elu chain and PSUM reads hide under
            # the next window's gathers and msg-multiplies.
            with (
                tc.tile_pool(name="g1", bufs=3) as g1p,
                tc.tile_pool(name="gd1", bufs=3) as gd1p,
                tc.tile_pool(name="al1", bufs=3) as al1p,
                tc.tile_pool(name="wend", bufs=2) as wendp,
                tc.tile_pool(name="l2h", bufs=3) as l2hp,
                tc.tile_pool(name="gd2", bufs=1) as gd2p,
                tc.tile_pool(name="ps1", bufs=2, space="PSUM") as ps1p,
                tc.tile_pool(name="l2ps", bufs=2, space="PSUM") as l2ps,
                tc.tile_pool(name="l2tp", bufs=2, space="PSUM") as l2tp,
            ):
                # remaining L2 metadata on SP early in L1
                nc.sync.dma_start(srcg2[:], srcg2_d[:])
                nc.sync.dma_start(dstg2[:], dstg2_d[:])
                nc.sync.dma_start(ewsb[:], ewc2_d[:])

                gd2all = gd2p.tile([128, NB, NCH, KC, T2W], FP)

                def chunks(w, psU, psD):
                    nc.sync.dma_start(selsb[:, w], selT_d[:, w])
                    for ch in range(NCH):
                        g = g1p.tile([128, KC, TMW], FP)
                        nc.gpsimd.dma_gather(
                            g[:], tableM[:], srcg[:, w, ch, :], SL, SL, TMW
                        )
                        gd = gd1p.tile([128, KC, 64], FP)
                        nc.gpsimd.dma_gather(
                            gd[:], tableM[:, 256:320], dstg[:, w, ch, :], SL, SL, 64,
                            elem_step=TMW,
                        )
                        gb = g[:, :, 256:264].bitcast(BF)    # [128, KC, 16]
                        gdb = gd[:, :, 0:8].bitcast(BF)      # [128, KC, 16]
                        a = al1p.tile([128, KC, 8], FP)
                        nc.gpsimd.tensor_tensor(
                            out=a[:], in0=gb[:, :, 0:8], in1=gdb[:, :, 8:16],
                            op=mybir.AluOpType.add,
                        )
                        nc.gpsimd.tensor_tensor(
                            out=a[:], in0=a[:], in1=ewc8sb[:, w, ch],
                            op=mybir.AluOpType.add,
                        )
                        nc.vector.scalar_tensor_tensor(
                            out=a[:], in0=a[:], scalar=0.2, in1=a[:],
                            op0=mybir.AluOpType.mult, op1=mybir.AluOpType.max)
                        ahb = al1p.tile([128, KC, 1, 8], BF)
                        nc.scalar.activation(ahb[:, :, 0, :], a[:],
                                             mybir.ActivationFunctionType.Exp)
                        mv = g[:, :, 0:256].bitcast(BF)
                        msg4 = mv.rearrange("p t (c h) -> p t c h", h=8)
                        ah4 = ahb[:].to_broadcast([128, KC, 64, 8])
                        nc.vector.tensor_tensor(out=msg4, in0=msg4, in1=ah4,
                                                op=mybir.AluOpType.mult)
                        for t in range(KC):
                            ti = ch * KC + t
                            st = ti == 0
                            sp = ti == K - 1
                            sel = selsb[:, w, ch, t * 128 : (t + 1) * 128]
                            nc.tensor.matmul(psU[:], sel, g[:, t, 0:256].bitcast(BF),
                                             start=st, stop=sp)
                            nc.tensor.matmul(psD[:], sel, ahb[:, t, 0, :],
                                             start=st, stop=sp)

                def fin1(w, psU, psD):
                    # h1 = U/D in bf16 (DVE only)
                    dr = wendp.tile([128, 1, 8], FP, tag="dr")
                    nc.vector.reciprocal(dr[:, 0, :], psD[:])
                    h1v = h1own[:, w, :]
                    h1v3 = h1v.rearrange("p (c h) -> p c h", h=8)
                    psU3 = psU[:].rearrange("p (c h) -> p c h", h=8)
                    nc.vector.tensor_tensor(out=h1v3, in0=psU3,
                                            in1=dr[:].to_broadcast([128, 64, 8]),
                                            op=mybir.AluOpType.mult)
                    if _build_program.use_b1:
                        nc.vector.tensor_tensor(out=h1v, in0=h1v, in1=b1rep[:],
                                                op=mybir.AluOpType.add)

                def finA(w):
                    # ACT elu chain pieces
                    h1v = h1own[:, w, :]
                    negp = wendp.tile([128, C1], BF, tag="negp")
                    nc.scalar.activation(negp[:], h1v,
                                         mybir.ActivationFunctionType.Relu,
                                         scale=-1.0)
                    emin = wendp.tile([128, C1], BF, tag="emin")
                    nc.scalar.activation(emin[:], negp[:],
                                         mybir.ActivationFunctionType.Exp,
                                         scale=-1.0)
                    posp = wendp.tile([128, C1], BF, tag="posp")
                    nc.scalar.activation(posp[:], h1v,
                                         mybir.ActivationFunctionType.Relu)
                    return posp, emin

                def fin2(w, posp, emin):
                    # elu combine + layer-2 projection + dst-side L2 gathers
                    h1v = h1own[:, w, :]
                    nc.vector.scalar_tensor_tensor(
                        out=h1v, in0=posp[:], scalar=-1.0, in1=emin[:],
                        op0=mybir.AluOpType.add, op1=mybir.AluOpType.add,
                    )
                    ps2 = l2ps.tile([128, 8], FP)
                    for kc in range(4):
                        tps = l2tp.tile([128, 128], BF)
                        nc.tensor.transpose(
                            tps[:], h1own[:, w, kc * 128 : (kc + 1) * 128], identb[:])
                        tsb = l2hp.tile([128, 128], BF, tag="tsb")
                        nc.scalar.copy(tsb[:], tps[:])
                        nc.tensor.matmul(ps2[:], tsb[:], W2sb[:, kc, :],
                                         start=(kc == 0), stop=(kc == 3))
                    # bf16-packed row: [4 h2 | s2src | s2dst | pad] = 16B
                    st2b = l2hp.tile([128, 8], BF, tag="st2b")
                    nc.scalar.copy(st2b[:], ps2[:])
                    st2f = st2b[:].bitcast(FP)              # [128, 4]
                    nc.sync.dma_start(table2c[w * 128 : (w + 1) * 128, :], st2f)
                    out2x = AP(table2x[:].tensor, w * 128 * T2W,
                               [(T2W, 128), (1, 4)])
                    nc.sync.dma_start(out2x, st2f)
                    for ch in range(NCH):
                        nc.gpsimd.dma_gather(
                            gd2all[:, w, ch], table2x[:], dstg2[:, w, ch, :],
                            SL, SL, T2W,
                        )

                hist = {}
                for w in range(NB):
                    psU = ps1p.tile([128, 512], FP, tag="psU")
                    psD = ps1p.tile([128, 8], FP, tag="psD")
                    if w >= 1:
                        fin1(w - 1, *hist[w - 1][:2])
                    chunks(w, psU, psD)
                    if w >= 1:
                        hist[w - 1] = hist[w - 1][:2] + (finA(w - 1),)
                    if w >= 2:
                        fin2(w - 2, *hist.pop(w - 2)[2])
                    hist[w] = (psU, psD)
                fin1(NB - 1, *hist[NB - 1][:2])
                hist[NB - 1] = hist[NB - 1][:2] + (finA(NB - 1),)
                fin2(NB - 2, *hist.pop(NB - 2)[2])
                fin2(NB - 1, *hist.pop(NB - 1)[2])

                # single AllGather of the bf16-packed compact table (16B rows)
                nc.gpsimd.collective_compute(
                    "AllGather", mybir.AluOpType.bypass,
                    replica_groups=[list(range(NCORES))],
                    ins=[table2c[:]],
                    outs=[t2cf[:]],
                )
                # expansion: drop rows into cols 0:4 of table2f
                nc.sync.dma_start(table2f[:, 0:4], t2cf[:])

            # ================= layer 2 edge phase =================
            with (
                tc.tile_pool(name="g2", bufs=3) as g2p,
                tc.tile_pool(name="al2", bufs=3) as al2p,
                tc.tile_pool(name="wend2", bufs=2) as wend2p,
                tc.tile_pool(name="ps2p", bufs=2, space="PSUM") as ps2pp,
            ):
                for w in range(NB):
                    psO = ps2pp.tile([128, 8], FP)
                    for ch in range(NCH):
                        gs = g2p.tile([128, KC, T2W], FP)
                        nc.gpsimd.dma_gather(
                            gs[:], table2f[:], srcg2[:, w, ch, :], SL, SL, T2W
                        )
                        gsb = gs[:, :, 0:4].bitcast(BF)      # [128, KC, 8]
                        gdb2 = gd2all[:, w, ch, :, 2:4].bitcast(BF)  # [128, KC, 4]
                        a2 = al2p.tile([128, KC, 1], FP, tag="a2")
                        nc.vector.tensor_tensor(out=a2[:], in0=gsb[:, :, 4:5],
                                                in1=gdb2[:, :, 1:2],
                                                op=mybir.AluOpType.add)
                        ewc2_b = ewsb[:, w, ch, :].to_broadcast([128, KC, 1])
                        nc.vector.tensor_tensor(out=a2[:], in0=a2[:], in1=ewc2_b,
                                                op=mybir.AluOpType.add)
                        nc.vector.scalar_tensor_tensor(
                            out=a2[:], in0=a2[:], scalar=0.2, in1=a2[:],
                            op0=mybir.AluOpType.mult, op1=mybir.AluOpType.max)
                        g5b = al2p.tile([128, KC, 8], BF, tag="g5b")
                        nc.vector.tensor_copy(g5b[:, :, 0:4], gsb[:, :, 0:4])
                        nc.scalar.activation(g5b[:, :, 4:5], a2[:],
                                             mybir.ActivationFunctionType.Exp)
                        ah = g5b[:, :, 4:5].to_broadcast([128, KC, 4])
                        nc.vector.tensor_tensor(out=g5b[:, :, 0:4],
                                                in0=g5b[:, :, 0:4],
                                                in1=ah, op=mybir.AluOpType.mult)
                        for t in range(KC):
                            ti = ch * KC + t
                            sel = selsb[:, w, ch, t * 128 : (t + 1) * 128]
                            nc.tensor.matmul(psO[:, 0:5], sel, g5b[:, t, 0:5],
                                             start=(ti == 0), stop=(ti == K - 1))
                    dr = wend2p.tile([128, 1], FP, tag="dr2")
                    nc.vector.reciprocal(dr[:], psO[:, 4:5])
                    ob = wend2p.tile([128, 8], FP, tag="ob")
                    nc.vector.tensor_tensor(out=ob[:, 0:4], in0=psO[:, 0:4],
                                            in1=dr[:].to_broadcast([128, 4]),
                                            op=mybir.AluOpType.mult)
                    if _build_program.use_b2:
                        nc.vector.tensor_tensor(out=ob[:, 0:4], in0=ob[:, 0:4],
                                                in1=b2rep[:, 0:4],
                                                op=mybir.AluOpType.add)
                    nc.sync.dma_start(out_d[w * 128 : (w + 1) * 128, :], ob[:, 0:4])

    nc.compile()
    return nc


def _balance_windows(dst):
    """Greedy in-degree balancing of nodes into NWIN windows of WIN slots."""
    import heapq

    indeg = np.bincount(dst, minlength=N)
    order = np.argsort(-indeg, kind="stable")
    heap = [(0, w) for w in range(NWIN)]
    heapq.heapify(heap)
    fill = np.zeros(NWIN, np.int64)
    node_win = np.zeros(N, np.int64)
    node_slot = np.zeros(N, np.int64)
    for n in order:
        cnt, w = heapq.heappop(heap)
        node_win[n] = w
        node_slot[n] = fill[w]
        fill[w] += 1
        if fill[w] < WIN:
            heapq.heappush(heap, (cnt + int(indeg[n]), w))
    return node_win, node_slot


def _prepare(x, edge_index, edge_weight, W1, att_src1, att_dst1, att_edge1, We1, b1,
             W2, att_src2, att_dst2, att_edge2, We2, b2):
    x = np.asarray(x, np.float32)
    ei = np.asarray(edge_index)
    ew = np.asarray(edge_weight, np.float32)
    W1 = np.asarray(W1, np.float32)
    att_src1 = np.asarray(att_src1, np.float32)
    att_dst1 = np.asarray(att_dst1, np.float32)
    att_edge1 = np.asarray(att_edge1, np.float32)
    We1 = np.asarray(We1, np.float32)
    b1 = np.asarray(b1, np.float32)
    W2 = np.asarray(W2, np.float32)
    att_src2 = np.asarray(att_src2, np.float32)
    att_dst2 = np.asarray(att_dst2, np.float32)
    att_edge2 = np.asarray(att_edge2, np.float32)
    We2 = np.asarray(We2, np.float32)
    b2 = np.asarray(b2, np.float32)

    # ---------- weight folding ----------
    W1r = W1.reshape(IN, H, HID)
    Wsrc = np.einsum("khc,hc->kh", W1r, att_src1)
    Wdst = np.einsum("khc,hc->kh", W1r, att_dst1)
    c1 = (We1.reshape(H, HID) * att_edge1).sum(1).astype(np.float32)  # [H]

    # head-minor column order: new col c*8+h = old h*64+c
    cols = np.tile(np.arange(H), HID) * HID + np.repeat(np.arange(HID), H)
    W1p = np.ascontiguousarray(W1[:, cols])
    b1p = b1[cols]
    W2p = W2[cols, :]
    W1sd = np.concatenate([W1p, Wsrc, Wdst], axis=1)  # [IN, 528]

    Waug2 = np.zeros((C1, 8), np.float32)
    Waug2[:, 0:4] = W2p
    Waug2[:, 4] = W2p @ att_src2[0]
    Waug2[:, 5] = W2p @ att_dst2[0]
    W2resh = np.ascontiguousarray(Waug2.reshape(4, 128, 8).transpose(1, 0, 2))
    c2 = float((We2[0] * att_edge2[0]).sum())
    _build_program.c2_host = c2
    _build_program.use_b1 = bool(np.any(b1))
    _build_program.use_b2 = bool(np.any(b2))

    # ---------- edge partitioning ----------
    src = np.asarray(ei[0], np.int64)
    dst = np.asarray(ei[1], np.int64)

    node_win, node_slot = _balance_windows(dst)
    node_core = node_win // NB
    node_w = node_win % NB
    node_local = node_w * WIN + node_slot
    node_gpad = node_core * NPAD + node_local

    ekey = node_win[dst]
    order = np.argsort(ekey, kind="stable")
    s_s, d_s, w_s = src[order], dst[order], ew[order]
    core_of = node_core[d_s]
    win_of = node_w[d_s]
    loc_of = node_slot[d_s]

    cnt = np.bincount(node_win[d_s], minlength=NWIN)
    K = int(np.ceil(cnt.max() / 128.0))
    NCHo = os.environ.get("BASS_GAT_NCH")
    if NCHo is not None:
        NCH = int(NCHo)
        KC = (K + NCH - 1) // NCH
    else:
        NCH = 2
        while ((K + NCH - 1) // NCH) * 128 > MAXI:
            NCH += 1
        KC = (K + NCH - 1) // NCH
    K = KC * NCH
    SL = KC * 128
    SW = K * 128

    in_maps = []
    base_rep = {
        "xTb": np.ascontiguousarray(x.T).astype(ml_dtypes.bfloat16),
        "W1sd": W1sd.astype(ml_dtypes.bfloat16),
        "W2r": W2resh.astype(ml_dtypes.bfloat16),
        "b2rep": np.tile(np.concatenate([b2, np.zeros(4, np.float32)])[None, :],
                         (128, 1)),
        "b1rep": np.tile(b1p[None, :], (128, 1)),
        "identb": np.eye(128, dtype=np.float32).astype(ml_dtypes.bfloat16),
    }

    for c in range(NCORES):
        m = dict(base_rep)
        srcgm = np.zeros((NB, NCH, 128, KC * 8), np.int16)
        dstgm = np.zeros((NB, NCH, 128, KC * 8), np.int16)
        srcg2m = np.zeros((NB, NCH, 128, KC * 8), np.int16)
        dstg2m = np.zeros((NB, NCH, 128, KC * 8), np.int16)
        ews = np.zeros((NB, NCH, KC, 128), np.float32)
        selTm = np.zeros((NB, NCH, 128, SL), np.float32)
        sel_c = core_of == c
        for w in range(NB):
            es = np.nonzero(sel_c & (win_of == w))[0]
            ns = len(es)
            ssrc = np.zeros(SW, np.int64)
            sdst = np.zeros(SW, np.int64)
            sew = np.zeros(SW, np.float32)
            sloc = np.full(SW, -1, np.int64)
            ssrc[:ns] = s_s[es]
            sdst[:ns] = d_s[es]
            sew[:ns] = w_s[es]
            sloc[:ns] = loc_of[es]
            for ch in range(NCH):
                sl = slice(ch * SL, (ch + 1) * SL)
                srcgm[w, ch] = _wrap_idx(ssrc[sl])
                dstgm[w, ch] = _wrap_idx(sdst[sl])
                srcg2m[w, ch] = _wrap_idx(node_gpad[ssrc[sl]])
                dstg2m[w, ch] = _wrap_idx(node_local[sdst[sl]])
                ews[w, ch] = sew[sl].reshape(KC, 128)
                lc = sloc[sl]
                valid = np.nonzero(lc >= 0)[0]
                tt, pp = valid // 128, valid % 128
                selTm[w, ch, pp, tt * 128 + lc[valid]] = 1.0
        m["srcg"] = np.ascontiguousarray(srcgm.transpose(2, 0, 1, 3))
        m["dstg"] = np.ascontiguousarray(dstgm.transpose(2, 0, 1, 3))
        m["srcg2"] = np.ascontiguousarray(srcg2m.transpose(2, 0, 1, 3))
        m["dstg2"] = np.ascontiguousarray(dstg2m.transpose(2, 0, 1, 3))
        ewt = np.ascontiguousarray(ews.transpose(3, 0, 1, 2))
        m["ewc2"] = ewt * c2
        m["ewc8"] = np.ascontiguousarray(ewt[..., None] * c1)
        m["selT"] = np.ascontiguousarray(
            selTm.transpose(2, 0, 1, 3)).astype(ml_dtypes.bfloat16)
        in_maps.append(m)

    meta = (node_core, node_local)
    return in_maps, KC, NCH, c2, meta


def kernel(**inputs):
    global LAST_EXEC_NS, LAST_RESULTS
    in_maps, KC, NCH, c2, meta = _prepare(**inputs)
    key = (KC, NCH, c2, _build_program.use_b1, _build_program.use_b2)
    if key not in _CACHE:
        _CACHE[key] = _build_program(KC, NCH)
    nc = _CACHE[key]

    trace = os.environ.get("BASS_GAT_TRACE", "0") == "1"
    res = run_bass_kernel_spmd(nc, in_maps, list(range(NCORES)), trace=trace)
    LAST_EXEC_NS = res.exec_time_ns
    LAST_RESULTS = res
    node_core, node_local = meta
    per_core = [res.results[c]["out_own"] for c in range(NCORES)]
    out = np.empty((N, 4), np.float32)
    for c in range(NCORES):
        mask = node_core == c
        out[mask] = per_core[c][node_local[mask]]
    return out


# revision 15
# speedup vs baseline: 1.5251x; 1.0177x over previous
"""Trainium2 Bass kernel for a 2-layer GAT (nn_GAT_781684048444).

Strategy (8 NeuronCores, SPMD) — v2:
  - Nodes assigned to 80 windows (8 cores x 10 windows x 128 slots) by greedy
    in-degree balancing; edges grouped by dst window, padded to K 128-edge
    tiles; one static program serves all cores. Output inverse-permuted.
  - Stage 0 (replicated): ONE bf16 matmul per 128-node block computes the
    packed row [512 bf16 msg (head-minor) | 8 fp32 s_src | 8 fp32 s_dst]
    (attention dots folded into the weight matrix; the fp32 PSUM result is
    staged to SBUF once, msg cols reach DRAM through a gpsimd cast-DMA
    (fp32->bf16 in the DMA), scalar cols through a raw bit-copy DMA on SP).
    No separate fp32 x load, no second matmul, no big DVE copy traffic.
  - Layer-1 edge phase: gather 1280B rows by src + 256B scalar blocks by dst;
    alpha adds run on gpsimd, leaky-relu on DVE, exp on ACT; the bf16
    msg *= alpha broadcast runs in DVE 2x mode; scatter-add via resident
    one-hot selT bf16 matmuls into PSUM. h1 = elu(U/D) with the elu expressed
    as relu/exp on ACT (elu(x) = relu(x) + exp(-relu(-x)) - 1), only the
    final combine on DVE. h1 kept bf16.
  - Layer-2 projection per window (bf16 transpose + matmul) feeds a compact
    [NPAD, 8] fp32 table; the AllGather is SPLIT in two (windows 0-4 issued
    mid-layer-1, 5-9 at the end) so most of its fixed cost overlaps layer 1.
    dst-side gathers + alpha partials are computed in the shadow of the
    second collective; src gathers + the final gather-attend-scatter follow.
"""

import os
import sys

import ml_dtypes
import numpy as np

sys.path.insert(0, "/opt/trn_rl_repo")

from concourse import bacc, bass, mybir, tile  # noqa: E402
from concourse.bass import AP  # noqa: E402
from concourse.bass_utils import run_bass_kernel_spmd  # noqa: E402

N, E = 10000, 160000
IN, HID, OUT, H = 128, 64, 4, 8
C1 = H * HID               # 512 layer-1 out width
TMW = 320                  # fp32 row: 256 (512 bf16 msg) | 8 ssrc | 8 sdst | 48 pad
T2W = 64                   # layer-2 expanded row width (fp32)
NCORES = 8
NPC = N // NCORES
WIN = 128
NB = 10
NPAD = NB * WIN            # 1280
NWIN = NCORES * NB         # 80
NBLK = (N + 127) // 128    # 79
MAXI = 1024
WSPLIT = 5                 # windows in first collective slice

FP = mybir.dt.float32
BF = mybir.dt.bfloat16
I16 = mybir.dt.int16

_CACHE = {}

LAST_EXEC_NS = None
LAST_RESULTS = None


def _wrap_idx(vals):
    """int16 gather index layout: idx i -> [i%16, i//16], tiled to 128 partitions."""
    n = vals.shape[0]
    w = np.zeros((16, n // 16), np.int16)
    w[np.arange(n) % 16, np.arange(n) // 16] = vals.astype(np.int16)
    return np.tile(w, (8, 1))


def _build_program(KC, NCH):
    K = KC * NCH
    SL = KC * 128

    nc = bacc.Bacc("TRN2", target_bir_lowering=False, debug=False, num_devices=NCORES)

    # ---- DRAM parameters ----
    xTb_d = nc.dram_tensor("xTb", [IN, N], BF, kind="ExternalInput")
    W1sd_d = nc.dram_tensor("W1sd", [IN, 528], BF, kind="ExternalInput")
    W2_d = nc.dram_tensor("W2r", [128, 4, 8], BF, kind="ExternalInput")
    b2_d = nc.dram_tensor("b2rep", [128, 8], FP, kind="ExternalInput")
    b1_d = nc.dram_tensor("b1rep", [128, C1], FP, kind="ExternalInput")
    ident_d = nc.dram_tensor("identb", [128, 128], BF, kind="ExternalInput")
    ewc8_d = nc.dram_tensor("ewc8", [128, NB, NCH, KC, 8], FP, kind="ExternalInput")
    ewc2_d = nc.dram_tensor("ewc2", [128, NB, NCH, KC], FP, kind="ExternalInput")
    srcg_d = nc.dram_tensor("srcg", [128, NB, NCH, KC * 8], I16, kind="ExternalInput")
    dstg_d = nc.dram_tensor("dstg", [128, NB, NCH, KC * 8], I16, kind="ExternalInput")
    srcg2_d = nc.dram_tensor("srcg2", [128, NB, NCH, KC * 8], I16, kind="ExternalInput")
    dstg2_d = nc.dram_tensor("dstg2", [128, NB, NCH, KC * 8], I16, kind="ExternalInput")
    selT_d = nc.dram_tensor("selT", [128, NB, NCH, SL], BF, kind="ExternalInput")

    out_d = nc.dram_tensor("out_own", [NPAD, 4], FP, kind="ExternalOutput")

    # ---- internal DRAM ----
    tableM = nc.dram_tensor("tableM", [NBLK * 128, TMW], FP)
    table2x = nc.dram_tensor("table2x", [NPAD, T2W], FP)
    table2c = nc.dram_tensor("table2c", [NPAD, 4], FP)
    t2cf = nc.dram_tensor("t2cf", [NCORES * NPAD, 4], FP, addr_space="Shared")
    table2f = nc.dram_tensor("table2f", [NCORES * NPAD, T2W], FP)

    c2_host = _build_program.c2_host
    XCH = 10

    with tile.TileContext(nc) as tc:
        with (
            tc.tile_pool(name="const", bufs=1) as constp,
            tc.tile_pool(name="idx", bufs=1) as idxp,
            tc.tile_pool(name="h1p", bufs=1) as h1p,
            tc.tile_pool(name="selp", bufs=1) as selp,
        ):
            W1sd = constp.tile([IN, 528], BF)
            nc.scalar.dma_start(W1sd[:], W1sd_d[:])
            W2sb = constp.tile([128, 4, 8], BF)
            nc.scalar.dma_start(W2sb[:], W2_d[:])
            b2rep = constp.tile([128, 8], FP)
            nc.scalar.dma_start(b2rep[:], b2_d[:])
            if _build_program.use_b1:
                b1rep = constp.tile([128, C1], FP)
                nc.scalar.dma_start(b1rep[:], b1_d[:])
            identb = constp.tile([128, 128], BF)
            nc.scalar.dma_start(identb[:], ident_d[:])
            ewc8sb = constp.tile([128, NB, NCH, KC, 8], FP)
            ewsb = constp.tile([128, NB, NCH, KC], FP)
            srcg = idxp.tile([128, NB, NCH, KC * 8], I16)
            dstg = idxp.tile([128, NB, NCH, KC * 8], I16)
            srcg2 = idxp.tile([128, NB, NCH, KC * 8], I16)
            dstg2 = idxp.tile([128, NB, NCH, KC * 8], I16)
            selsb = selp.tile([128, NB, NCH, SL], BF)
            h1own = h1p.tile([128, NB, C1], BF)

            # ========== stage 0: packed table via cast-DMA ==========
            with (
                tc.tile_pool(name="s0x", bufs=8) as s0xp,
                tc.tile_pool(name="s0stg", bufs=3) as s0stgp,
                tc.tile_pool(name="s0ps", bufs=4, space="PSUM") as s0ps,
            ):
                xbch = []
                for xc in range((NBLK + XCH - 1) // XCH):
                    c0 = xc * XCH * 128
                    cz = min(N, (xc + 1) * XCH * 128)
                    xb = s0xp.tile([IN, XCH * 128], BF, tag="xb")
                    nc.sync.dma_start(xb[:, : cz - c0], xTb_d[:, c0:cz])
                    xbch.append(xb)
                # L1 metadata queues behind x on SP
                nc.sync.dma_start(srcg[:], srcg_d[:])
                nc.sync.dma_start(dstg[:], dstg_d[:])
                nc.sync.dma_start(ewc8sb[:], ewc8_d[:])

                stg = None
                for b in range(NBLK):
                    rows = min(128, N - b * 128)
                    off = (b % XCH) * 128
                    psAB = s0ps.tile([128, 1024], FP, tag="psAB")
                    xsl = xbch[b // XCH][:, off : off + rows]
                    nc.tensor.matmul(psAB[:rows, 0:512], xsl, W1sd[:, 0:512],
                                     start=True, stop=True)
                    nc.tensor.matmul(psAB[:rows, 512:528], xsl, W1sd[:, 512:528],
                                     start=True, stop=True)
                    if b % 4 == 0:
                        stg = s0stgp.tile([128, 4, 528], FP, tag="stg")
                    bi = b % 4
                    if b == NBLK - 1 and rows < 128:
                        nc.vector.memset(stg[:, bi, :], 0.0)
                    if b % 2 == 0:
                        nc.vector.tensor_copy(stg[:rows, bi, :], psAB[:rows, 0:528])
                    else:
                        nc.scalar.copy(stg[:rows, bi, :], psAB[:rows, 0:528])
                    if bi == 3 or b == NBLK - 1:
                        gsz = bi + 1
                        b0 = b - bi
                        outM = AP(tableM[:].tensor, b0 * 128 * TMW,
                                  [(TMW, 128), (128 * TMW, gsz), (1, 264)]).bitcast(BF)
                        nc.gpsimd.dma_start(outM, stg[:, 0:gsz, 0:528])

            # ================= layer 1 edge phase =================
            # Software-pipelined: window w's finalize is split around window
            # w+1's chunk work so the ACT elu chain and PSUM reads hide under
            # the next window's gathers and msg-multiplies.
            with (
                tc.tile_pool(name="g1", bufs=3) as g1p,
                tc.tile_pool(name="gd1", bufs=3) as gd1p,
                tc.tile_pool(name="al1", bufs=3) as al1p,
                tc.tile_pool(name="wend", bufs=2) as wendp,
                tc.tile_pool(name="l2h", bufs=3) as l2hp,
                tc.tile_pool(name="gd2", bufs=1) as gd2p,
                tc.tile_pool(name="ps1", bufs=2, space="PSUM") as ps1p,
                tc.tile_pool(name="l2ps", bufs=2, space="PSUM") as l2ps,
                tc.tile_pool(name="l2tp", bufs=2, space="PSUM") as l2tp,
            ):
                # remaining L2 metadata on SP early in L1
                nc.sync.dma_start(srcg2[:], srcg2_d[:])
                nc.sync.dma_start(dstg2[:], dstg2_d[:])
                nc.sync.dma_start(ewsb[:], ewc2_d[:])

                gd2all = gd2p.tile([128, NB, NCH, KC, T2W], FP)
                a2pall = gd2p.tile([128, NB, NCH, KC, 1], FP)

                def chunks(w, psU, psD):
                    nc.sync.dma_start(selsb[:, w], selT_d[:, w])
                    for ch in range(NCH):
                        g = g1p.tile([128, KC, TMW], FP)
                        nc.gpsimd.dma_gather(
                            g[:], tableM[:], srcg[:, w, ch, :], SL, SL, TMW
                        )
                        gd = gd1p.tile([128, KC, 64], FP)
                        nc.gpsimd.dma_gather(
                            gd[:], tableM[:, 256:320], dstg[:, w, ch, :], SL, SL, 64,
                            elem_step=TMW,
                        )
                        gb = g[:, :, 256:264].bitcast(BF)    # [128, KC, 16]
                        gdb = gd[:, :, 0:8].bitcast(BF)      # [128, KC, 16]
                        a = al1p.tile([128, KC, 8], FP)
                        nc.gpsimd.tensor_tensor(
                            out=a[:], in0=gb[:, :, 0:8], in1=gdb[:, :, 8:16],
                            op=mybir.AluOpType.add,
                        )
                        nc.gpsimd.tensor_tensor(
                            out=a[:], in0=a[:], in1=ewc8sb[:, w, ch],
                            op=mybir.AluOpType.add,
                        )
                        nc.vector.scalar_tensor_tensor(
                            out=a[:], in0=a[:], scalar=0.2, in1=a[:],
                            op0=mybir.AluOpType.mult, op1=mybir.AluOpType.max)
                        ahb = al1p.tile([128, KC, 1, 8], BF)
                        nc.scalar.activation(ahb[:, :, 0, :], a[:],
                                             mybir.ActivationFunctionType.Exp)
                        mv = g[:, :, 0:256].bitcast(BF)
                        msg4 = mv.rearrange("p t (c h) -> p t c h", h=8)
                        ah4 = ahb[:].to_broadcast([128, KC, 64, 8])
                        nc.vector.tensor_tensor(out=msg4, in0=msg4, in1=ah4,
                                                op=mybir.AluOpType.mult)
                        for t in range(KC):
                            ti = ch * KC + t
                            st = ti == 0
                            sp = ti == K - 1
                            sel = selsb[:, w, ch, t * 128 : (t + 1) * 128]
                            nc.tensor.matmul(psU[:], sel, g[:, t, 0:256].bitcast(BF),
                                             start=st, stop=sp)
                            nc.tensor.matmul(psD[:], sel, ahb[:, t, 0, :],
                                             start=st, stop=sp)

                def fin1(w, psU, psD):
                    # h1 = U/D in bf16 (DVE only)
                    dr = wendp.tile([128, 1, 8], FP, tag="dr")
                    nc.vector.reciprocal(dr[:, 0, :], psD[:])
                    h1v = h1own[:, w, :]
                    h1v3 = h1v.rearrange("p (c h) -> p c h", h=8)
                    psU3 = psU[:].rearrange("p (c h) -> p c h", h=8)
                    nc.vector.tensor_tensor(out=h1v3, in0=psU3,
                                            in1=dr[:].to_broadcast([128, 64, 8]),
                                            op=mybir.AluOpType.mult)
                    if _build_program.use_b1:
                        nc.vector.tensor_tensor(out=h1v, in0=h1v, in1=b1rep[:],
                                                op=mybir.AluOpType.add)

                def finA(w):
                    # ACT elu chain pieces
                    h1v = h1own[:, w, :]
                    negp = wendp.tile([128, C1], BF, tag="negp")
                    nc.scalar.activation(negp[:], h1v,
                                         mybir.ActivationFunctionType.Relu,
                                         scale=-1.0)
                    emin = wendp.tile([128, C1], BF, tag="emin")
                    nc.scalar.activation(emin[:], negp[:],
                                         mybir.ActivationFunctionType.Exp,
                                         scale=-1.0)
                    posp = wendp.tile([128, C1], BF, tag="posp")
                    nc.scalar.activation(posp[:], h1v,
                                         mybir.ActivationFunctionType.Relu)
                    return posp, emin

                def fin2(w, posp, emin):
                    # elu combine + layer-2 projection + dst-side L2 gathers
                    h1v = h1own[:, w, :]
                    nc.vector.scalar_tensor_tensor(
                        out=h1v, in0=posp[:], scalar=-1.0, in1=emin[:],
                        op0=mybir.AluOpType.add, op1=mybir.AluOpType.add,
                    )
                    ps2 = l2ps.tile([128, 8], FP)
                    for kc in range(4):
                        tps = l2tp.tile([128, 128], BF)
                        nc.tensor.transpose(
                            tps[:], h1own[:, w, kc * 128 : (kc + 1) * 128], identb[:])
                        tsb = l2hp.tile([128, 128], BF, tag="tsb")
                        nc.scalar.copy(tsb[:], tps[:])
                        nc.tensor.matmul(ps2[:], tsb[:], W2sb[:, kc, :],
                                         start=(kc == 0), stop=(kc == 3))
                    # bf16-packed row: [4 h2 | s2src | s2dst | pad] = 16B
                    st2b = l2hp.tile([128, 8], BF, tag="st2b")
                    nc.scalar.copy(st2b[:], ps2[:])
                    st2f = st2b[:].bitcast(FP)              # [128, 4]
                    nc.sync.dma_start(table2c[w * 128 : (w + 1) * 128, :], st2f)
                    out2x = AP(table2x[:].tensor, w * 128 * T2W,
                               [(T2W, 128), (1, 4)])
                    nc.sync.dma_start(out2x, st2f)
                    for ch in range(NCH):
                        nc.gpsimd.dma_gather(
                            gd2all[:, w, ch], table2x[:], dstg2[:, w, ch, :],
                            SL, SL, T2W,
                        )

                hist = {}
                for w in range(NB):
                    psU = ps1p.tile([128, 512], FP, tag="psU")
                    psD = ps1p.tile([128, 8], FP, tag="psD")
                    if w == 0:
                        # PE p-state warmup during the first window's gathers;
                        # the real accumulation resets psU via start=True
                        for _ in range(22):
                            nc.tensor.matmul(psU[:], identb[:],
                                             W1sd[:, 0:512],
                                             start=True, stop=True)
                    if w >= 1:
                        fin1(w - 1, *hist[w - 1][:2])
                    chunks(w, psU, psD)
                    if w >= 1:
                        hist[w - 1] = hist[w - 1][:2] + (finA(w - 1),)
                    if w >= 2:
                        fin2(w - 2, *hist.pop(w - 2)[2])
                    hist[w] = (psU, psD)
                fin1(NB - 1, *hist[NB - 1][:2])
                hist[NB - 1] = hist[NB - 1][:2] + (finA(NB - 1),)
                fin2(NB - 2, *hist.pop(NB - 2)[2])
                fin2(NB - 1, *hist.pop(NB - 1)[2])

                # single AllGather of the bf16-packed compact table (16B rows)
                nc.gpsimd.collective_compute(
                    "AllGather", mybir.AluOpType.bypass,
                    replica_groups=[list(range(NCORES))],
                    ins=[table2c[:]],
                    outs=[t2cf[:]],
                )
                # dst-side alpha partials in the collective's shadow (DVE idle)
                for w in range(NB):
                    for ch in range(NCH):
                        gdb2 = gd2all[:, w, ch, :, 2:4].bitcast(BF)
                        ewc2_b = ewsb[:, w, ch, :].to_broadcast([128, KC, 1])
                        nc.vector.tensor_tensor(out=a2pall[:, w, ch],
                                                in0=gdb2[:, :, 1:2], in1=ewc2_b,
                                                op=mybir.AluOpType.add)
                # expansion: drop rows into cols 0:4 of table2f
                nc.sync.dma_start(table2f[:, 0:4], t2cf[:])

            # ================= layer 2 edge phase =================
            with (
                tc.tile_pool(name="g2", bufs=3) as g2p,
                tc.tile_pool(name="al2", bufs=3) as al2p,
                tc.tile_pool(name="wend2", bufs=2) as wend2p,
                tc.tile_pool(name="ps2p", bufs=2, space="PSUM") as ps2pp,
            ):
                for w in range(NB):
                    psO = ps2pp.tile([128, 8], FP)
                    for ch in range(NCH):
                        gs = g2p.tile([128, KC, T2W], FP)
                        nc.gpsimd.dma_gather(
                            gs[:], table2f[:], srcg2[:, w, ch, :], SL, SL, T2W
                        )
                        gsb = gs[:, :, 0:4].bitcast(BF)      # [128, KC, 8]
                        a2 = al2p.tile([128, KC, 1], FP, tag="a2")
                        nc.vector.tensor_tensor(out=a2[:], in0=gsb[:, :, 4:5],
                                                in1=a2pall[:, w, ch],
                                                op=mybir.AluOpType.add)
                        nc.vector.scalar_tensor_tensor(
                            out=a2[:], in0=a2[:], scalar=0.2, in1=a2[:],
                            op0=mybir.AluOpType.mult, op1=mybir.AluOpType.max)
                        g5b = al2p.tile([128, KC, 8], BF, tag="g5b")
                        nc.vector.tensor_copy(g5b[:, :, 0:4], gsb[:, :, 0:4])
                        nc.scalar.activation(g5b[:, :, 4:5], a2[:],
                                             mybir.ActivationFunctionType.Exp)
                        ah = g5b[:, :, 4:5].to_broadcast([128, KC, 4])
                        nc.vector.tensor_tensor(out=g5b[:, :, 0:4],
                                                in0=g5b[:, :, 0:4],
                                                in1=ah, op=mybir.AluOpType.mult)
                        for t in range(KC):
                            ti = ch * KC + t
                            sel = selsb[:, w, ch, t * 128 : (t + 1) * 128]
                            nc.tensor.matmul(psO[:, 0:5], sel, g5b[:, t, 0:5],
                                             start=(ti == 0), stop=(ti == K - 1))
                    dr = wend2p.tile([128, 1], FP, tag="dr2")
                    nc.vector.reciprocal(dr[:], psO[:, 4:5])
                    ob = wend2p.tile([128, 8], FP, tag="ob")
                    nc.vector.tensor_tensor(out=ob[:, 0:4], in0=psO[:, 0:4],
                                            in1=dr[:].to_broadcast([128, 4]),
                                            op=mybir.AluOpType.mult)
                    if _build_program.use_b2:
                        nc.vector.tensor_tensor(out=ob[:, 0:4], in0=ob[:, 0:4],
                                                in1=b2rep[:, 0:4],
                                                op=mybir.AluOpType.add)
                    nc.sync.dma_start(out_d[w * 128 : (w + 1) * 128, :], ob[:, 0:4])

    nc.compile()
    return nc


def _balance_windows(dst):
    """Greedy in-degree balancing of nodes into NWIN windows of WIN slots."""
    import heapq

    indeg = np.bincount(dst, minlength=N)
    order = np.argsort(-indeg, kind="stable")
    heap = [(0, w) for w in range(NWIN)]
    heapq.heapify(heap)
    fill = np.zeros(NWIN, np.int64)
    node_win = np.zeros(N, np.int64)
    node_slot = np.zeros(N, np.int64)
    for n in order:
        cnt, w = heapq.heappop(heap)
        node_win[n] = w
        node_slot[n] = fill[w]
        fill[w] += 1
        if fill[w] < WIN:
            heapq.heappush(heap, (cnt + int(indeg[n]), w))
    return node_win, node_slot


def _prepare(x, edge_index, edge_weight, W1, att_src1, att_dst1, att_edge1, We1, b1,
             W2, att_src2, att_dst2, att_edge2, We2, b2):
    x = np.asarray(x, np.float32)
    ei = np.asarray(edge_index)
    ew = np.asarray(edge_weight, np.float32)
    W1 = np.asarray(W1, np.float32)
    att_src1 = np.asarray(att_src1, np.float32)
    att_dst1 = np.asarray(att_dst1, np.float32)
    att_edge1 = np.asarray(att_edge1, np.float32)
    We1 = np.asarray(We1, np.float32)
    b1 = np.asarray(b1, np.float32)
    W2 = np.asarray(W2, np.float32)
    att_src2 = np.asarray(att_src2, np.float32)
    att_dst2 = np.asarray(att_dst2, np.float32)
    att_edge2 = np.asarray(att_edge2, np.float32)
    We2 = np.asarray(We2, np.float32)
    b2 = np.asarray(b2, np.float32)

    # ---------- weight folding ----------
    W1r = W1.reshape(IN, H, HID)
    Wsrc = np.einsum("khc,hc->kh", W1r, att_src1)
    Wdst = np.einsum("khc,hc->kh", W1r, att_dst1)
    c1 = (We1.reshape(H, HID) * att_edge1).sum(1).astype(np.float32)  # [H]

    # head-minor column order: new col c*8+h = old h*64+c
    cols = np.tile(np.arange(H), HID) * HID + np.repeat(np.arange(HID), H)
    W1p = np.ascontiguousarray(W1[:, cols])
    b1p = b1[cols]
    W2p = W2[cols, :]
    W1sd = np.concatenate([W1p, Wsrc, Wdst], axis=1)  # [IN, 528]

    Waug2 = np.zeros((C1, 8), np.float32)
    Waug2[:, 0:4] = W2p
    Waug2[:, 4] = W2p @ att_src2[0]
    Waug2[:, 5] = W2p @ att_dst2[0]
    W2resh = np.ascontiguousarray(Waug2.reshape(4, 128, 8).transpose(1, 0, 2))
    c2 = float((We2[0] * att_edge2[0]).sum())
    _build_program.c2_host = c2
    _build_program.use_b1 = bool(np.any(b1))
    _build_program.use_b2 = bool(np.any(b2))

    # ---------- edge partitioning ----------
    src = np.asarray(ei[0], np.int64)
    dst = np.asarray(ei[1], np.int64)

    node_win, node_slot = _balance_windows(dst)
    node_core = node_win // NB
    node_w = node_win % NB
    node_local = node_w * WIN + node_slot
    node_gpad = node_core * NPAD + node_local

    ekey = node_win[dst]
    order = np.argsort(ekey, kind="stable")
    s_s, d_s, w_s = src[order], dst[order], ew[order]
    core_of = node_core[d_s]
    win_of = node_w[d_s]
    loc_of = node_slot[d_s]

    cnt = np.bincount(node_win[d_s], minlength=NWIN)
    K = int(np.ceil(cnt.max() / 128.0))
    NCHo = os.environ.get("BASS_GAT_NCH")
    if NCHo is not None:
        NCH = int(NCHo)
        KC = (K + NCH - 1) // NCH
    else:
        NCH = 2
        while ((K + NCH - 1) // NCH) * 128 > MAXI:
            NCH += 1
        KC = (K + NCH - 1) // NCH
    K = KC * NCH
    SL = KC * 128
    SW = K * 128

    in_maps = []
    base_rep = {
        "xTb": np.ascontiguousarray(x.T).astype(ml_dtypes.bfloat16),
        "W1sd": W1sd.astype(ml_dtypes.bfloat16),
        "W2r": W2resh.astype(ml_dtypes.bfloat16),
        "b2rep": np.tile(np.concatenate([b2, np.zeros(4, np.float32)])[None, :],
                         (128, 1)),
        "b1rep": np.tile(b1p[None, :], (128, 1)),
        "identb": np.eye(128, dtype=np.float32).astype(ml_dtypes.bfloat16),
    }

    for c in range(NCORES):
        m = dict(base_rep)
        srcgm = np.zeros((NB, NCH, 128, KC * 8), np.int16)
        dstgm = np.zeros((NB, NCH, 128, KC * 8), np.int16)
        srcg2m = np.zeros((NB, NCH, 128, KC * 8), np.int16)
        dstg2m = np.zeros((NB, NCH, 128, KC * 8), np.int16)
        ews = np.zeros((NB, NCH, KC, 128), np.float32)
        selTm = np.zeros((NB, NCH, 128, SL), np.float32)
        sel_c = core_of == c
        for w in range(NB):
            es = np.nonzero(sel_c & (win_of == w))[0]
            ns = len(es)
            ssrc = np.zeros(SW, np.int64)
            sdst = np.zeros(SW, np.int64)
            sew = np.zeros(SW, np.float32)
            sloc = np.full(SW, -1, np.int64)
            ssrc[:ns] = s_s[es]
            sdst[:ns] = d_s[es]
            sew[:ns] = w_s[es]
            sloc[:ns] = loc_of[es]
            for ch in range(NCH):
                sl = slice(ch * SL, (ch + 1) * SL)
                srcgm[w, ch] = _wrap_idx(ssrc[sl])
                dstgm[w, ch] = _wrap_idx(sdst[sl])
                srcg2m[w, ch] = _wrap_idx(node_gpad[ssrc[sl]])
                dstg2m[w, ch] = _wrap_idx(node_local[sdst[sl]])
                ews[w, ch] = sew[sl].reshape(KC, 128)
                lc = sloc[sl]
                valid = np.nonzero(lc >= 0)[0]
                tt, pp = valid // 128, valid % 128
                selTm[w, ch, pp, tt * 128 + lc[valid]] = 1.0
        m["srcg"] = np.ascontiguousarray(srcgm.transpose(2, 0, 1, 3))
        m["dstg"] = np.ascontiguousarray(dstgm.transpose(2, 0, 1, 3))
        m["srcg2"] = np.ascontiguousarray(srcg2m.transpose(2, 0, 1, 3))
        m["dstg2"] = np.ascontiguousarray(dstg2m.transpose(2, 0, 1, 3))
        ewt = np.ascontiguousarray(ews.transpose(3, 0, 1, 2))
        m["ewc2"] = ewt * c2
        m["ewc8"] = np.ascontiguousarray(ewt[..., None] * c1)
        m["selT"] = np.ascontiguousarray(
            selTm.transpose(2, 0, 1, 3)).astype(ml_dtypes.bfloat16)
        in_maps.append(m)

    meta = (node_core, node_local)
    return in_maps, KC, NCH, c2, meta


def kernel(**inputs):
    global LAST_EXEC_NS, LAST_RESULTS
    in_maps, KC, NCH, c2, meta = _prepare(**inputs)
    key = (KC, NCH, c2, _build_program.use_b1, _build_program.use_b2)
    if key not in _CACHE:
        _CACHE[key] = _build_program(KC, NCH)
    nc = _CACHE[key]

    trace = os.environ.get("BASS_GAT_TRACE", "0") == "1"
    res = run_bass_kernel_spmd(nc, in_maps, list(range(NCORES)), trace=trace)
    LAST_EXEC_NS = res.exec_time_ns
    LAST_RESULTS = res
    node_core, node_local = meta
    per_core = [res.results[c]["out_own"] for c in range(NCORES)]
    out = np.empty((N, 4), np.float32)
    for c in range(NCORES):
        mask = node_core == c
        out[mask] = per_core[c][node_local[mask]]
    return out
